# revision 1
# baseline (speedup 1.0000x reference)
"""Trainium2 Bass kernel for nn_DocREModel (8-core SPMD).

Sharding: data-parallel over the 4 documents x 2 pair-halves = 8 cores.
Each core runs an identical program; per-core behavior differs only via
its input data (its doc's tensors + its half of the pair one-hots).

All floating-point arithmetic runs on device. Host does only index-driven
data movement: batch slicing, transposes, row gathers at integer indices,
and one-hot/selector matrix construction.
"""

import numpy as np
from contextlib import ExitStack

import concourse.bass as bass
import concourse.bacc as bacc
import concourse.tile as tile
import concourse.mybir as mybir
from concourse.bass_utils import run_bass_kernel_spmd

FP32 = mybir.dt.float32
BF16 = mybir.dt.bfloat16

# compute dtypes per stage
SEQ_DT = BF16    # seq transform matmuls
CONV_DT = BF16   # conv stack
PAIR_DT = BF16   # pair-classification matmuls
GRAPH_DT = BF16  # rgcn / entity-attention matmuls

import ml_dtypes

_NPDT = {FP32: np.float32, BF16: ml_dtypes.bfloat16}

B, C, H, NH = 4, 1024, 768, 12
E, M, L, LS = 22, 3, 30, 16
NN, NF, EMB = 118, 532, 512
P, PH = 462, 231
IC = 256
S = 22            # spatial side of relation map
SP = S * S        # 484
PW = 32           # padded side (32 for aligned rows)
SPP = PW * PW     # 676
ACT = mybir.ActivationFunctionType
KT_H = H // 128   # 6
ATTM_ROWS = E * M * NH          # 792
ATTM_TILES = [128] * 6 + [24]
SPAN_ROWS = L * LS              # 480
SPAN_TILES = [128, 128, 128, 96]
NF_TILES = [128, 128, 128, 128, 20]   # 532
SP_TILES = [128, 128, 128, 100]       # 484


def _ts(sizes):
    """(offset, size) pairs for a tiling."""
    off = 0
    for sz in sizes:
        yield off, sz
        off += sz


def build_program():
    nc = bacc.Bacc("TRN2", target_bir_lowering=False, debug=False)

    dins = {}

    def din(name, shape, dt=FP32):
        dins[name] = nc.dram_tensor(name, shape, dt, kind="ExternalInput").ap()
        return dins[name]

    xt = din("xt", [H, C], SEQ_DT)            # X.T
    xg = din("xg", [H, E * M], SEQ_DT)        # X rows at mention idx, transposed
    xspan = din("xspan", [SPAN_ROWS, H], SEQ_DT)  # X rows at span positions
    attm = din("attm", [ATTM_ROWS, C], GRAPH_DT)  # attention mention rows, (e,m)*12+h major
    attl = din("attl", [SPAN_ROWS, NH * LS], FP32)  # link blocks, row l*16+j, free (h,i)
    adjt = din("adjt", [NN, 5 * NN], FP32)    # 4 relations + identity (self)
    typ = din("typ", [NN, 20], GRAPH_DT)      # type_embed[node_types]
    wtrans = din("wtrans", [H, EMB], SEQ_DT)
    btrans = din("btrans", [1, EMB], FP32)
    gmat = din("gmat", [ATTM_ROWS, E], GRAPH_DT)  # kron(I22, ones(36))/36
    g3 = din("g3", [E * M, E], FP32)          # kron(I22, ones(3))
    gspan = din("gspan", [SPAN_ROWS, L], SEQ_DT)  # kron(I30, ones(16))
    ones = din("ones", [128, 1], FP32)
    ident = din("ident", [128, 128], FP32)
    identp = din("identp", [128, 128], PAIR_DT)
    wrel = din("wrel", [5 * NF, EMB], GRAPH_DT)   # rows r*532+k, r=4 is W_self
    brgcn = din("brgcn", [EMB, 1], FP32)
    w1t = din("w1t", [25 * EMB, IC], CONV_DT)   # rows tap*512+ic
    b1 = din("b1", [IC, 1], FP32)
    w2t = din("w2t", [25 * IC, IC], CONV_DT)
    b2 = din("b2", [IC, 1], FP32)
    w3t = din("w3t", [25 * IC, EMB], CONV_DT)
    b3 = din("b3", [EMB, 1], FP32)
    sh = din("sh", [E, PH], FP32)
    st = din("st", [E, PH], FP32)
    sm = din("sm", [SP, PH], PAIR_DT)
    wht = din("wht", [4 * EMB, 2 * EMB], PAIR_DT)
    bht = din("bht", [2 * EMB, 1], FP32)
    wbil = din("wbil", [2 * EMB, 97], PAIR_DT)
    bbil = din("bbil", [97, 1], FP32)
    outt = nc.dram_tensor("outt", [97, PH], FP32, kind="ExternalOutput").ap()

    with tile.TileContext(nc) as tc, ExitStack() as ctx:
        pp = ctx.enter_context(tc.tile_pool(name="persist", bufs=1))
        pst = ctx.enter_context(tc.tile_pool(name="stream", bufs=1))
        pps = ctx.enter_context(tc.tile_pool(name="psum", bufs=8, space="PSUM"))
        pdram = ctx.enter_context(tc.tile_pool(name="dram", bufs=1, space="DRAM"))


        dma = nc.sync.dma_start

        def T(pool, shape, dt, tag, bufs=None):
            return pool.tile(shape, dt, tag=tag, name=tag, bufs=bufs)


        # ---- persistent small tiles ----
        ident_t = T(pp, [128, 128], FP32, "ident")
        dma(ident_t[:], ident)
        identp_t = T(pp, [128, 128], PAIR_DT, "identp")
        dma(identp_t[:], identp)
        # ~4us of dummy matmuls during the initial input-DMA wait: flips the
        # PE HAM clock gate to 8/8 before the sparse graph phase begins
        ps_warm = T(pps, [128, 128], FP32, "ps")
        for _ in range(72):
            nc.tensor.matmul(ps_warm[:], identp_t[:], identp_t[:],
                             start=True, stop=True)
        warm_sb = T(pp, [128, 128], FP32, "warm_sb")
        nc.vector.tensor_copy(warm_sb[:], ps_warm[:])
        warm_dram = pdram.tile([128, 128], FP32, name="warm_dram")
        dma(warm_dram[:], warm_sb[:])
        ones_t = T(pp, [128, 1], FP32, "ones")
        dma(ones_t[:], ones)
        btrans_t = T(pp, [1, EMB], FP32, "btrans")
        dma(btrans_t[:], btrans)
        btrans_bc = T(pp, [128, EMB], FP32, "btrans_bc")
        nc.gpsimd.partition_broadcast(btrans_bc[:], btrans_t[:])
        nodes_e = T(pp, [E, NF], GRAPH_DT, "nodes_e")
        nodes_m = T(pp, [E * M, NF], GRAPH_DT, "nodes_m")
        nodes_l = T(pp, [L, NF], GRAPH_DT, "nodes_l")
        dma(nodes_e[:, EMB:NF], typ[0:E, :])
        dma(nodes_m[:, EMB:NF], typ[E:E + E * M, :])
        dma(nodes_l[:, EMB:NF], typ[E + E * M:NN, :])

        NODE_GROUPS = [(0, E), (E, E * M), (E + E * M, L)]
        node_tiles = [nodes_e, nodes_m, nodes_l]
        # adjacency normalize; cols 0:472 are the 4 relations, 472:590 identity
        adjt_t = T(pp, [NN, 5 * NN], FP32, "adjt")
        dma(adjt_t[:], adjt)
        ps_rsA = T(pps, [1, 4 * NN], FP32, "ps")
        nc.tensor.matmul(ps_rsA[:], ones_t[0:NN, 0:1], adjt_t[:, 0:4 * NN],
                         start=True, stop=True)
        ps_rsB = T(pps, [1, NN], FP32, "ps")
        nc.tensor.matmul(ps_rsB[:], ones_t[0:NN, 0:1], adjt_t[:, 4 * NN:5 * NN],
                         start=True, stop=True)
        rs_t = T(pp, [1, 5 * NN], FP32, "rs")
        nc.vector.tensor_scalar_add(rs_t[:, 0:4 * NN], ps_rsA[:], 1e-5)
        nc.vector.tensor_scalar_add(rs_t[:, 4 * NN:5 * NN], ps_rsB[:], 1e-5)
        rcp_t = T(pp, [1, 5 * NN], FP32, "rcp")
        nc.vector.reciprocal(rcp_t[:], rs_t[:])
        rsbc_t = T(pp, [128, 5 * NN], FP32, "rsbc")
        nc.gpsimd.partition_broadcast(rsbc_t[:], rcp_t[:])
        adjn_t = []
        for gi, (goff, gsz) in enumerate(NODE_GROUPS):
            tf = T(pst, [gsz, 5 * NN], FP32, "adjn_f32", bufs=3)
            dma(tf[:], adjt[goff:goff + gsz, :])
            t = T(pp, [gsz, 5 * NN], GRAPH_DT, f"adjn{gi}")
            nc.vector.tensor_mul(t[:], tf[:], rsbc_t[0:gsz, :])
            adjn_t.append(t)


        # ---- S1: seq = X @ W_trans + b  (natural layout [1024 tok, 512]) ----
        wtrans_t = []
        for kt in range(KT_H):
            t = T(pp, [128, EMB], SEQ_DT, f"wtrans{kt}")
            dma(t[:], wtrans[kt * 128:(kt + 1) * 128, :])
            wtrans_t.append(t)

        ps_seq = [T(pps, [128, EMB], FP32, "ps") for _ in range(8)]
        for kt in range(KT_H):
            xt_t = T(pst, [128, C], SEQ_DT, "xt_stream", bufs=2)
            dma(xt_t[:], xt[kt * 128:(kt + 1) * 128, :])
            for mt in range(8):
                nc.tensor.matmul(
                    ps_seq[mt][:], xt_t[:, mt * 128:(mt + 1) * 128], wtrans_t[kt][:],
                    start=(kt == 0), stop=(kt == KT_H - 1))
        seq_t = []
        for mt in range(8):
            t = T(pp, [128, EMB], SEQ_DT, f"seq{mt}")
            nc.vector.tensor_add(t[:], ps_seq[mt][:], btrans_bc[:])
            seq_t.append(t)

        # ---- S2: mention embeddings + entity logsumexp nodes ----
        ps_memb = T(pps, [E * M, EMB], FP32, "ps")
        for kt in range(KT_H):
            xg_t = T(pst, [128, E * M], SEQ_DT, "xg_stream", bufs=3)
            dma(xg_t[:], xg[kt * 128:(kt + 1) * 128, :])
            nc.tensor.matmul(ps_memb[:], xg_t[:], wtrans_t[kt][:],
                             start=(kt == 0), stop=(kt == KT_H - 1))
        memb_t = T(pp, [E * M, EMB], FP32, "memb")
        nc.vector.tensor_add(memb_t[:], ps_memb[:], btrans_bc[0:E * M, :])
        nc.vector.tensor_copy(nodes_m[:, 0:EMB], memb_t[:])
        ememb_t = T(pp, [E * M, EMB], FP32, "ememb")
        nc.scalar.activation(ememb_t[:], memb_t[:], ACT.Exp)
        g3_t = T(pp, [E * M, E], FP32, "g3")
        dma(g3_t[:], g3)
        ps_ent = T(pps, [E, EMB], FP32, "ps")
        nc.tensor.matmul(ps_ent[:], g3_t[:], ememb_t[:], start=True, stop=True)
        nc.scalar.activation(nodes_e[:, 0:EMB], ps_ent[:], ACT.Ln)

        # ---- S3: link nodes ----
        # a[s] = mean over (h,i) of the 16x16 link attention block, s=(l,j)
        aT_t, aTb_t, xspan_t, gspan_t = [], [], [], []
        for i, (off, sz) in enumerate(_ts(SPAN_TILES)):
            al = T(pst, [sz, NH * LS], FP32, "attl_stream", bufs=2)
            dma(al[:], attl[off:off + sz, :])
            a = T(pp, [sz, 1], FP32, f"aT{i}")
            nc.vector.tensor_reduce(a[:], al[:], mybir.AxisListType.X,
                                    mybir.AluOpType.add)
            nc.vector.tensor_scalar_mul(a[:], a[:], 1.0 / (NH * LS))
            aT_t.append(a)
            ab = T(pp, [sz, 1], SEQ_DT, f"aTb{i}")
            nc.vector.tensor_copy(ab[:], a[:])
            aTb_t.append(ab)
            gs = T(pp, [sz, L], SEQ_DT, f"gspan{i}")
            dma(gs[:], gspan[off:off + sz, :])
            gspan_t.append(gs)
            xs = T(pp, [sz, H], SEQ_DT, f"xspan{i}")
            dma(xs[:], xspan[off:off + sz, :])
            xspan_t.append(xs)
        # asum[l] = sum_j a_l[j] (for the bias term); uses unscaled-by-X a
        ps_as = T(pps, [L, 1], FP32, "ps")
        for kt in range(4):
            nc.tensor.matmul(ps_as[:], gspan_t[kt][:], aTb_t[kt][:],
                             start=(kt == 0), stop=(kt == 3))
        asum_t = T(pp, [L, 1], FP32, "asum")
        nc.vector.tensor_copy(asum_t[:], ps_as[:])
        # scale xspan rows by a in place, then project through gspan
        for kt in range(4):
            nc.vector.tensor_scalar_mul(xspan_t[kt][:], xspan_t[kt][:],
                                        aT_t[kt][:])
        # linkctxT [768, 30]
        lct_t = []
        for mt in range(KT_H):
            ps = T(pps, [128, L], FP32, "ps")
            for kt in range(4):
                nc.tensor.matmul(ps[:], xspan_t[kt][:, mt * 128:(mt + 1) * 128],
                                 gspan_t[kt][:], start=(kt == 0), stop=(kt == 3))
            t = T(pp, [128, L], SEQ_DT, f"lct{mt}")
            nc.vector.tensor_copy(t[:], ps[:])
            lct_t.append(t)
        bterm_t = T(pp, [L, EMB], FP32, "bterm")
        nc.vector.tensor_scalar_mul(bterm_t[:], btrans_bc[0:L, :], asum_t[:])
        ps_link = T(pps, [L, EMB], FP32, "ps")
        for kt in range(KT_H):
            nc.tensor.matmul(ps_link[:], lct_t[kt][:], wtrans_t[kt][:],
                             start=(kt == 0), stop=(kt == KT_H - 1))
        nc.vector.tensor_add(nodes_l[:, 0:EMB], ps_link[:], bterm_t[:])

        # ---- S4: ea (entity attention) + e_ctx ----
        ps_ea = [T(pps, [E, 512], FP32, "ps") for _ in range(2)]
        n_attm = len(ATTM_TILES)
        for i, (off, sz) in enumerate(_ts(ATTM_TILES)):
            at = T(pst, [sz, C], GRAPH_DT, "attm_stream", bufs=2)
            dma(at[:], attm[off:off + sz, :])
            gt = T(pst, [sz, E], GRAPH_DT, "gmat_stream", bufs=3)
            dma(gt[:], gmat[off:off + sz, :])
            for half in range(2):
                nc.tensor.matmul(ps_ea[half][:], gt[:],
                                 at[:, half * 512:(half + 1) * 512],
                                 start=(i == 0), stop=(i == n_attm - 1))
        ea_t = T(pp, [E, C], FP32, "ea")
        for half in range(2):
            nc.vector.tensor_copy(ea_t[:, half * 512:(half + 1) * 512],
                                  ps_ea[half][:])
        rsum_t = T(pp, [E, 1], FP32, "rsum")
        nc.vector.tensor_reduce(rsum_t[:], ea_t[:], mybir.AxisListType.X,
                                mybir.AluOpType.add)
        nc.vector.tensor_scalar_add(rsum_t[:], rsum_t[:], 1e-5)
        recip_t = T(pp, [E, 1], FP32, "recip")
        nc.vector.reciprocal(recip_t[:], rsum_t[:])
        ean_t = T(pp, [E, C], FP32, "ean")
        nc.vector.tensor_scalar_mul(ean_t[:], ea_t[:], recip_t[:])
        # eaNT via PE transpose, then e_ctx [22, 512]
        ps_ectx = T(pps, [E, EMB], FP32, "ps")
        for kt in range(8):
            pst_ea = T(pps, [128, E], FP32, "ps")
            nc.tensor.transpose(pst_ea[:], ean_t[:, kt * 128:(kt + 1) * 128],
                                ident_t[0:E, 0:E])
            eaT = T(pst, [128, E], SEQ_DT, "eaT_stream", bufs=3)
            nc.vector.tensor_copy(eaT[:], pst_ea[:])
            nc.tensor.matmul(ps_ectx[:], eaT[:], seq_t[kt][:],
                             start=(kt == 0), stop=(kt == 7))
        ectx_t = T(pp, [E, EMB], FP32, "ectx")
        nc.vector.tensor_copy(ectx_t[:], ps_ectx[:])

        def conv_pass(ps_c, in_tiles, w_dram, n_ic_t, n_oc_t, ocs,
                      first, last):
            """One accumulation pass of a 5x5 SAME conv on 22x22."""
            n_acc = 25 * n_ic_t
            a = 0
            for tap in range(25):
                di, dj = divmod(tap, 5)
                for kt in range(n_ic_t):
                    w = T(pst, [128, ocs], CONV_DT, "wconv_stream", bufs=16)
                    dma(w[:], w_dram[(tap * n_ic_t + kt) * 128:
                                     (tap * n_ic_t + kt + 1) * 128, :])
                    rhs = in_tiles[kt][:].rearrange(
                        "p (a b) -> p a b", a=PW, b=PW)[:, di:di + S, dj:dj + S]
                    for mt in range(n_oc_t):
                        nc.tensor.matmul(ps_c[mt][:], w[:, mt * 128:(mt + 1) * 128],
                                         rhs, start=(first and a == 0),
                                         stop=(last and a == n_acc - 1))
                    a += 1

        def conv(in_tiles, w_dram, n_ic_t, n_oc_t, ocs, bias_tiles, out_cb):
            ps_c = [T(pps, [128, SP], FP32, "ps") for _ in range(n_oc_t)]
            conv_pass(ps_c, in_tiles, w_dram, n_ic_t, n_oc_t, ocs, True, True)
            for mt in range(n_oc_t):
                out_cb(mt, ps_c[mt])

        # ---- S5: RGCN ----
        # msgT for all 5 relations at once (A: 4 real relations, B: self)
        msgA_t, msgB_t = [], []
        for i, (off, sz) in enumerate(_ts(NF_TILES)):
            psA = T(pps, [sz, 4 * NN], FP32, "ps")
            psB = T(pps, [sz, NN], FP32, "ps")
            for gi, (goff, gsz) in enumerate(NODE_GROUPS):
                nc.tensor.matmul(psA[:], node_tiles[gi][:, off:off + sz],
                                 adjn_t[gi][:, 0:4 * NN],
                                 start=(gi == 0), stop=(gi == 2))
                nc.tensor.matmul(psB[:], node_tiles[gi][:, off:off + sz],
                                 adjn_t[gi][:, 4 * NN:5 * NN],
                                 start=(gi == 0), stop=(gi == 2))
            tA = T(pp, [sz, 4 * NN], GRAPH_DT, f"msgA{i}")
            nc.vector.tensor_copy(tA[:], psA[:])
            msgA_t.append(tA)
            tB = T(pp, [sz, NN], GRAPH_DT, f"msgB{i}")
            nc.vector.tensor_copy(tB[:], psB[:])
            msgB_t.append(tB)

        ps_gcn = [T(pps, [128, NN], FP32, "ps") for _ in range(4)]
        n_terms = 5 * 5  # (4 rel + self) x 5 k-tiles
        term = 0
        for r in range(5):
            for i, (off, sz) in enumerate(_ts(NF_TILES)):
                w = T(pst, [sz, EMB], GRAPH_DT, "wg_stream", bufs=5)
                dma(w[:], wrel[r * NF + off:r * NF + off + sz, :])
                rhs = (msgA_t[i][:, r * NN:(r + 1) * NN] if r < 4
                       else msgB_t[i][:])
                for mt in range(4):
                    nc.tensor.matmul(ps_gcn[mt][:], w[:, mt * 128:(mt + 1) * 128],
                                     rhs, start=(term == 0),
                                     stop=(term == n_terms - 1))
                term += 1
        # [512,1] needs 4 partition tiles
        brgcn_tiles = []
        for mt in range(4):
            t = T(pp, [128, 1], FP32, f"brgcn{mt}")
            dma(t[:], brgcn[mt * 128:(mt + 1) * 128, :])
            brgcn_tiles.append(t)
        gcnT_t = []
        for mt in range(4):
            t = T(pp, [128, NN], FP32, f"gcnT{mt}")
            nc.scalar.activation(t[:], ps_gcn[mt][:], ACT.Relu,
                                 bias=brgcn_tiles[mt][:, 0:1])
            gcnT_t.append(t)
        # ent natural [22, 512]; entT view = gcnT[:, 0:22]
        ent_t = T(pp, [E, EMB], FP32, "ent")
        for mt in range(4):
            ps = T(pps, [E, 128], FP32, "ps")
            nc.tensor.transpose(ps[:], gcnT_t[mt][:, 0:E], ident_t[:, :])
            nc.vector.tensor_copy(ent_t[:, mt * 128:(mt + 1) * 128], ps[:])
        # ectxT tiles [128, 22] x4
        ectxT_t = []
        for mt in range(4):
            ps = T(pps, [128, E], FP32, "ps")
            nc.tensor.transpose(ps[:], ectx_t[:, mt * 128:(mt + 1) * 128],
                                ident_t[0:E, 0:E])
            t = T(pp, [128, E], FP32, f"ectxT{mt}")
            nc.vector.tensor_copy(t[:], ps[:])
            ectxT_t.append(t)

        # ---- S6: relation map x + conv stack ----
        xpad_t = []
        for mt in range(4):
            xp = T(pp, [128, SPP], CONV_DT, f"xpad{mt}")
            nc.vector.memset(xp[:], 0.0)
            entT_v = gcnT_t[mt][:, 0:E]
            t1 = T(pp, [128, SP], FP32, "xtmp1")
            nc.vector.tensor_mul(
                t1[:].rearrange("p (a b) -> p a b", a=S, b=S),
                entT_v.unsqueeze(2).to_broadcast((128, S, S)),
                entT_v.unsqueeze(1).to_broadcast((128, S, S)))
            t2 = T(pp, [128, SP], FP32, "xtmp2")
            nc.vector.tensor_mul(
                t2[:].rearrange("p (a b) -> p a b", a=S, b=S),
                ectxT_t[mt][:].unsqueeze(2).to_broadcast((128, S, S)),
                ectxT_t[mt][:].unsqueeze(1).to_broadcast((128, S, S)))
            inner = xp[:].rearrange("p (a b) -> p a b", a=PW, b=PW)[:, 2:2 + S, 2:2 + S]
            nc.vector.tensor_add(inner, t1[:], t2[:])
            xpad_t.append(xp)

        # conv1: 512 -> 256, output into padded tiles for conv2
        pad1_t = []
        for mt in range(2):
            t = T(pp, [128, SPP], CONV_DT, f"pad1_{mt}")
            nc.vector.memset(t[:], 0.0)
            pad1_t.append(t)
        b1_tiles = []
        for mt in range(2):
            t = T(pp, [128, 1], FP32, f"b1_{mt}")
            dma(t[:], b1[mt * 128:(mt + 1) * 128, :])
            b1_tiles.append(t)

        def c1_out(mt, ps):
            inner = pad1_t[mt][:].rearrange("p (a b) -> p a b", a=PW, b=PW)[
                :, 2:2 + S, 2:2 + S]
            nc.scalar.activation(inner, ps[:].rearrange("p (a b) -> p a b", a=S, b=S),
                                 ACT.Relu, bias=b1_tiles[mt][:, 0:1])

        conv(xpad_t, w1t, 4, 2, IC, b1_tiles, c1_out)

        pad2_t = []
        for mt in range(2):
            t = T(pp, [128, SPP], CONV_DT, f"pad2_{mt}")
            nc.vector.memset(t[:], 0.0)
            pad2_t.append(t)
        b2_tiles = []
        for mt in range(2):
            t = T(pp, [128, 1], FP32, f"b2_{mt}")
            dma(t[:], b2[mt * 128:(mt + 1) * 128, :])
            b2_tiles.append(t)

        def c2_out(mt, ps):
            inner = pad2_t[mt][:].rearrange("p (a b) -> p a b", a=PW, b=PW)[
                :, 2:2 + S, 2:2 + S]
            nc.scalar.activation(inner, ps[:].rearrange("p (a b) -> p a b", a=S, b=S),
                                 ACT.Relu, bias=b2_tiles[mt][:, 0:1])

        conv(pad1_t, w2t, 2, 2, IC, b2_tiles, c2_out)

        x3_t = []
        b3_tiles = []
        for mt in range(4):
            t = T(pp, [128, 1], FP32, f"b3_{mt}")
            dma(t[:], b3[mt * 128:(mt + 1) * 128, :])
            b3_tiles.append(t)
        for mt in range(4):
            t = T(pp, [128, SP], PAIR_DT, f"x3_{mt}")
            x3_t.append(t)

        def c3_out(mt, ps):
            nc.scalar.activation(x3_t[mt][:], ps[:], ACT.Relu,
                                 bias=b3_tiles[mt][:, 0:1])

        conv(pad2_t, w3t, 2, 4, EMB, b3_tiles, c3_out)

        # ---- S7: pair features + classifier ----
        # x3T [484, 512]
        x3T_t = []
        for i, (off, sz) in enumerate(_ts(SP_TILES)):
            t = T(pp, [sz, EMB], PAIR_DT, f"x3T{i}")
            x3T_t.append(t)
        for i, (off, sz) in enumerate(_ts(SP_TILES)):
            for src in range(4):
                ps = T(pps, [sz, 64], FP32, "ps")
                psb = ps[:].bitcast(PAIR_DT)
                nc.tensor.transpose(psb, x3_t[src][:, off:off + sz],
                                    identp_t[:, :])
                nc.vector.tensor_copy(x3T_t[i][:, src * 128:(src + 1) * 128], psb)

        sh_t = T(pp, [E, PH], FP32, "sh")
        dma(sh_t[:], sh)
        st_t = T(pp, [E, PH], FP32, "st")
        dma(st_t[:], st)
        sm_t = []
        for i, (off, sz) in enumerate(_ts(SP_TILES)):
            t = T(pp, [sz, PH], PAIR_DT, f"sm{i}")
            dma(t[:], sm[off:off + sz, :])
            sm_t.append(t)

        featT = [None] * 16
        for mt in range(4):
            ps = T(pps, [128, PH], FP32, "ps")
            nc.tensor.matmul(ps[:], ent_t[:, mt * 128:(mt + 1) * 128], sh_t[:],
                             start=True, stop=True)
            t = T(pp, [128, PH], PAIR_DT, f"featT{mt}")
            nc.vector.tensor_copy(t[:], ps[:])
            featT[mt] = t
        for mt in range(4):
            ps = T(pps, [128, PH], FP32, "ps")
            nc.tensor.matmul(ps[:], ent_t[:, mt * 128:(mt + 1) * 128], st_t[:],
                             start=True, stop=True)
            t = T(pp, [128, PH], PAIR_DT, f"featT{4 + mt}")
            nc.vector.tensor_copy(t[:], ps[:])
            featT[4 + mt] = t
        for mt in range(4):
            ps = T(pps, [128, PH], FP32, "ps")
            for i, (off, sz) in enumerate(_ts(SP_TILES)):
                nc.tensor.matmul(ps[:], x3T_t[i][:, mt * 128:(mt + 1) * 128],
                                 sm_t[i][:], start=(i == 0), stop=(i == 3))
            t = T(pp, [128, PH], PAIR_DT, f"featT{8 + mt}")
            nc.vector.tensor_copy(t[:], ps[:])
            featT[8 + mt] = t
        for mt in range(4):
            t = T(pp, [128, PH], PAIR_DT, f"featT{12 + mt}")
            nc.vector.tensor_mul(t[:], featT[mt][:], featT[4 + mt][:])
            featT[12 + mt] = t

        bht_tiles = []
        for mt in range(8):
            t = T(pp, [128, 1], FP32, f"bht{mt}")
            dma(t[:], bht[mt * 128:(mt + 1) * 128, :])
            bht_tiles.append(t)
        ps_ht = [T(pps, [128, PH], FP32, "ps") for _ in range(8)]
        for kt in range(16):
            w = T(pst, [128, 2 * EMB], PAIR_DT, "wht_stream", bufs=4)
            dma(w[:], wht[kt * 128:(kt + 1) * 128, :])
            for mt in range(8):
                nc.tensor.matmul(ps_ht[mt][:], w[:, mt * 128:(mt + 1) * 128],
                                 featT[kt][:], start=(kt == 0), stop=(kt == 15))
        htT_t = []
        for mt in range(8):
            t = T(pp, [128, PH], PAIR_DT, f"htT{mt}")
            nc.scalar.activation(t[:], ps_ht[mt][:], ACT.Tanh,
                                 bias=bht_tiles[mt][:, 0:1])
            htT_t.append(t)

        ps_out = T(pps, [97, PH], FP32, "ps")
        for kt in range(8):
            w = T(pst, [128, 97], PAIR_DT, "wbil_stream", bufs=3)
            dma(w[:], wbil[kt * 128:(kt + 1) * 128, :])
            nc.tensor.matmul(ps_out[:], w[:], htT_t[kt][:],
                             start=(kt == 0), stop=(kt == 7))
        bbil_t = T(pp, [97, 1], FP32, "bbil")
        dma(bbil_t[:], bbil)
        out_t = T(pp, [97, PH], FP32, "out")
        nc.vector.tensor_scalar_add(out_t[:], ps_out[:], bbil_t[:, 0:1])
        dma(outt, out_t[:])

    nc.compile()
    return nc


_PROG = None


def _get_prog():
    global _PROG
    if _PROG is None:
        _PROG = build_program()
    return _PROG


def _np(dt):
    return _NPDT[dt]


def _shared_inputs(inputs):
    f32 = np.float32
    sh = {}
    sh["wtrans"] = np.ascontiguousarray(inputs["W_trans"], _np(SEQ_DT))
    sh["btrans"] = np.ascontiguousarray(inputs["b_trans"], f32).reshape(1, EMB)
    sh["gmat"] = np.kron(np.eye(E, dtype=f32),
                         np.ones((M * NH, 1), f32) / (M * NH)).astype(_np(GRAPH_DT))
    sh["g3"] = np.kron(np.eye(E, dtype=f32), np.ones((M, 1), f32))
    sh["gspan"] = np.kron(np.eye(L, dtype=f32), np.ones((LS, 1), f32)).astype(_np(SEQ_DT))
    sh["ones"] = np.ones((128, 1), f32)
    sh["ident"] = np.eye(128, dtype=f32)
    sh["identp"] = np.eye(128, dtype=_np(PAIR_DT))
    sh["wrel"] = np.ascontiguousarray(np.concatenate(
        [np.asarray(inputs["W_rel"], f32).reshape(4 * NF, EMB),
         np.asarray(inputs["W_self"], f32)], axis=0)).astype(_np(GRAPH_DT))
    sh["brgcn"] = np.ascontiguousarray(inputs["b_rgcn"], f32).reshape(EMB, 1)
    sh["w1t"] = np.ascontiguousarray(
        np.asarray(inputs["conv1_w"], f32).transpose(2, 3, 1, 0).reshape(25 * EMB, IC),
        _np(CONV_DT))
    sh["b1"] = np.ascontiguousarray(inputs["conv1_b"], f32).reshape(IC, 1)
    sh["w2t"] = np.ascontiguousarray(
        np.asarray(inputs["conv2_w"], f32).transpose(2, 3, 1, 0).reshape(25 * IC, IC),
        _np(CONV_DT))
    sh["b2"] = np.ascontiguousarray(inputs["conv2_b"], f32).reshape(IC, 1)
    sh["w3t"] = np.ascontiguousarray(
        np.asarray(inputs["conv3_w"], f32).transpose(2, 3, 1, 0).reshape(25 * IC, EMB),
        _np(CONV_DT))
    sh["b3"] = np.ascontiguousarray(inputs["conv3_b"], f32).reshape(EMB, 1)
    sh["wht"] = np.ascontiguousarray(inputs["ht_W"], _np(PAIR_DT))
    sh["bht"] = np.ascontiguousarray(inputs["ht_b"], f32).reshape(2 * EMB, 1)
    sh["wbil"] = np.ascontiguousarray(inputs["bil_W"], _np(PAIR_DT))
    sh["bbil"] = np.ascontiguousarray(inputs["bil_b"], f32).reshape(97, 1)
    return sh


def _core_inputs(inputs, shared, b, hh):
    f32 = np.float32
    X = np.asarray(inputs["sequence_output"][b], f32)
    att = np.asarray(inputs["attention"][b], f32)
    adj = np.asarray(inputs["adjacency"][b], f32)
    mf = np.asarray(inputs["mention_idx"][b]).reshape(-1).astype(np.int64)
    ls = np.asarray(inputs["link_start"][b]).reshape(-1).astype(np.int64)
    ntypes = np.asarray(inputs["node_types"][b]).astype(np.int64)
    hts = np.asarray(inputs["hts"][b]).astype(np.int64)

    m = dict(shared)
    m["xt"] = np.ascontiguousarray(X.T, _np(SEQ_DT))
    m["xg"] = np.ascontiguousarray(X[mf].T, _np(SEQ_DT))
    pos = ls[:, None] + np.arange(LS)
    m["xspan"] = np.ascontiguousarray(X[pos.reshape(-1)]).astype(_np(SEQ_DT))
    rows = att[:, mf, :]
    m["attm"] = np.ascontiguousarray(
        rows.transpose(1, 0, 2).reshape(ATTM_ROWS, C)).astype(_np(GRAPH_DT))
    attl = np.empty((SPAN_ROWS, NH * LS), f32)
    for l in range(L):
        blk = att[:, pos[l], :][:, :, pos[l]]           # [12, 16i, 16j]
        attl[l * LS:(l + 1) * LS, :] = blk.transpose(2, 0, 1).reshape(LS, NH * LS)
    m["attl"] = attl
    m["adjt"] = np.ascontiguousarray(np.concatenate(
        [adj[r].T for r in range(4)] + [np.eye(NN, dtype=f32)], axis=1), f32)
    m["typ"] = np.ascontiguousarray(
        np.asarray(inputs["type_embed"], f32)[ntypes]).astype(_np(GRAPH_DT))
    pr = hts[hh * PH:(hh + 1) * PH]
    shm = np.zeros((E, PH), f32)
    shm[pr[:, 0], np.arange(PH)] = 1.0
    stm = np.zeros((E, PH), f32)
    stm[pr[:, 1], np.arange(PH)] = 1.0
    smm = np.zeros((SP, PH), f32)
    smm[pr[:, 0] * E + pr[:, 1], np.arange(PH)] = 1.0
    m["sh"] = shm
    m["st"] = stm
    m["sm"] = np.ascontiguousarray(smm, _np(PAIR_DT))
    return m


def kernel(**inputs):
    nc = _get_prog()
    shared = _shared_inputs(inputs)
    in_maps = []
    for b in range(B):
        for hh in range(2):
            in_maps.append(_core_inputs(inputs, shared, b, hh))
    res = run_bass_kernel_spmd(nc, in_maps, core_ids=list(range(8)))
    out = np.empty((B, P, 97), np.float32)
    for b in range(B):
        for hh in range(2):
            out[b, hh * PH:(hh + 1) * PH, :] = np.asarray(
                res.results[2 * b + hh]["outt"], np.float32).T
    return out



# revision 3
# speedup vs baseline: 1.5781x; 1.5781x over previous
"""Trainium2 Bass kernel for nn_DocREModel (8-core SPMD), v2.

Sharding: 4 docs x 2 half-bands = 8 cores. Each core runs an identical
program. The pair of cores sharing a doc splits the conv stack spatially:
each computes conv3 output rows 0..10 in ITS OWN entity coordinates.
Odd cores see the document with entity order reflected (e -> 21-e) and
spatially-flipped conv kernels, which makes "rows 0..10 local" equal to
global rows 11..21 — so both halves run the same compiled program.

Host does only index-driven data movement: batch slicing, transposes,
row gathers at integer indices, one-hot/selector construction, dtype
casts, and packing into wide-row DMA blobs. All FP arithmetic runs on
device.
"""

import numpy as np
from contextlib import ExitStack

import concourse.bass as bass
import concourse.bacc as bacc
import concourse.tile as tile
import concourse.mybir as mybir
from concourse.bass_utils import run_bass_kernel_spmd

import ml_dtypes

FP32 = mybir.dt.float32
BF16 = mybir.dt.bfloat16
_NPDT = {FP32: np.float32, BF16: ml_dtypes.bfloat16}

B, C, H, NH = 4, 1024, 768, 12
E, M, L, LS = 22, 3, 30, 16
NN, NF, EMB = 118, 532, 512
P, PH2, IC = 462, 256, 256
S = 22
ACT = mybir.ActivationFunctionType

# conv split geometry (local coords; identical on every core)
NX, N1, N2, N3 = 17, 15, 13, 11      # x rows 0..16 -> c1 0..14 -> c2 0..12 -> c3 0..10
PW = 26                               # padded cols (-2..23)
SP3 = N3 * S                          # 242 spatial positions of x3
SP3_TILES = [(0, 128), (128, SP3 - 128)]   # 128 + 114

ATTM_TILES = 7                        # 792 rows -> 7x128 (pad)
SPAN_TILES = 4                        # 480 rows -> 4x128 (pad)
GM_TILES = 7

# ---- C0..C4 blob16 layouts: (name, rows, cols[, count]) ----
def _layout(entries):
    off, d = 0, {}
    for name, cols in entries:
        d[name] = off
        off += cols
    return d, off

NODE_GROUPS = [22, 66, 30]            # entities, mentions, links

C0_L, C0_W = _layout([
    ("adjrel0", 110), ("adjrel1", 110), ("adjrel2", 110),
    ("typ0", 20), ("typ1", 20), ("typ2", 20),
    ("g3", 22), ("ones16", 1), ("gmat", GM_TILES * 22),
    ("gspan", SPAN_TILES * 30), ("attl", SPAN_TILES * 192),
    ("xg", 6 * 66), ("wtrans", 6 * 512),
])
C4_L, C4_W = _layout([("sh", PH2), ("st", PH2), ("sm", 2 * PH2)])
C1_W = 4 * 768        # xspan
C2_W = 7 * 1024       # attm
C3_W = 8 * 768        # xn

WCH = 13 * 512            # unified weight-chunk width (6656 cols bf16)

# blob32 layout
B32_L, B32_W = _layout([
    ("ident", 128), ("ones", 1), ("onesrow", 128), ("btrans", 512),
    ("brgcn", 512), ("b1", 2), ("b2", 2), ("b3", 4), ("bht", 8), ("bbil", 1),
])


def build_program():
    nc = bacc.Bacc("TRN2", target_bir_lowering=False, debug=False)

    def din(name, shape, dt=BF16):
        return nc.dram_tensor(name, shape, dt, kind="ExternalInput").ap()

    identp_d = din("identp", [128, 128])
    c0_d = din("c0", [128, C0_W])
    c1_d = din("c1", [128, C1_W])
    c2_d = din("c2", [128, 8192])
    c3_d = din("c3", [128, C3_W])
    c4_d = din("c4", [128, C4_W])
    b32_d = din("b32", [128, B32_W], FP32)
    wrel_d = din("wrel", [128, 2 * WCH])
    w1_d = din("w1", [128, 4 * WCH])
    w2_d = din("w2", [128, 2 * WCH])
    w3_d = din("w3", [128, 4 * WCH])
    wht_d = din("wht", [128, 16 * 1024])
    wbil_d = din("wbil", [128, 8 * 97])
    outt = nc.dram_tensor("outt", [97, PH2], FP32, kind="ExternalOutput").ap()

    with tile.TileContext(nc) as tc, ExitStack() as ctx:
        pp = ctx.enter_context(tc.tile_pool(name="persist", bufs=1))
        pst = ctx.enter_context(tc.tile_pool(name="stream", bufs=1))
        pps = ctx.enter_context(tc.tile_pool(name="psum", bufs=8, space="PSUM"))
        pdram = ctx.enter_context(tc.tile_pool(name="dram", bufs=1, space="DRAM"))

        dma = nc.sync.dma_start
        gdma = nc.gpsimd.dma_start

        def T(pool, shape, dt, tag, bufs=None):
            return pool.tile(shape, dt, tag=tag, name=tag, bufs=bufs)

        # ---- DMAs (order within each ring matters) ----
        identp_t = T(pp, [128, 128], BF16, "identp")
        dma(identp_t[:], identp_d)
        b32_t = T(pp, [128, B32_W], FP32, "b32")
        dma(b32_t[:], b32_d)
        c0_t = T(pp, [128, C0_W], BF16, "c0")
        dma(c0_t[:], c0_d)
        c3_t = T(pp, [128, C3_W], BF16, "c3")
        dma(c3_t[:], c3_d)
        # attm (2 chunks) shares a [128, 4096] ring with wht (4 chunks)
        c2_t = []
        for k in range(2):
            t = T(pst, [128, 4096], BF16, "b4k", bufs=2)
            dma(t[:], c2_d[:, k * 4096:(k + 1) * 4096])
            c2_t.append(t)
        c1_t = T(pp, [128, C1_W], BF16, "c1")
        dma(c1_t[:], c1_d)
        c4_t = T(pp, [128, C4_W], BF16, "c4")
        dma(c4_t[:], c4_d)
        # unified weight-chunk ring: wrel(2), w1(4), w2(2), w3(4)
        def wchunk(dram, k, engine_dma):
            t = T(pst, [128, WCH], BF16, "wchunk", bufs=4)
            engine_dma(t[:], dram[:, k * WCH:(k + 1) * WCH])
            return t

        wrel_t = [wchunk(wrel_d, k, gdma) for k in range(2)]
        w1_t = [wchunk(w1_d, k, gdma) for k in range(4)]
        w2_t = [wchunk(w2_d, k, gdma) for k in range(2)]
        w3_t = [wchunk(w3_d, k, dma) for k in range(4)]
        wht_t = []
        for k in range(4):
            t = T(pst, [128, 4096], BF16, "b4k", bufs=2)
            dma(t[:], wht_d[:, k * 4096:(k + 1) * 4096])
            wht_t.append(t)
        wbil_t = T(pp, [128, 8 * 97], BF16, "wbil")
        dma(wbil_t[:], wbil_d)

        def c0s(name, r0, r1, w):
            return c0_t[r0:r1, C0_L[name]:C0_L[name] + w]

        ident = b32_t[:, B32_L["ident"]:B32_L["ident"] + 128]
        onesrow = b32_t[0:1, B32_L["onesrow"]:B32_L["onesrow"] + 128]

        # ---- warmup matmuls: flip PE HAM to 8/8 during initial DMA wait ----
        ps_warm = T(pps, [128, 128], FP32, "ps")
        for _ in range(72):
            nc.tensor.matmul(ps_warm[:], identp_t[:], identp_t[:],
                             start=True, stop=True)
        warm_sb = T(pp, [128, 128], FP32, "warm_sb")
        nc.vector.tensor_copy(warm_sb[:], ps_warm[:])
        warm_dram = pdram.tile([128, 128], FP32, name="warm_dram")
        dma(warm_dram[:], warm_sb[:])

        # ---- broadcast rows via K=1 matmuls ----
        def bcast_row(src_row_ap, w, tag):
            ps = T(pps, [128, w], FP32, "ps")
            nc.tensor.matmul(ps[:], onesrow, src_row_ap, start=True, stop=True)
            t = T(pp, [128, w], FP32, tag)
            nc.vector.tensor_copy(t[:], ps[:])
            return t

        btrans_bc = bcast_row(b32_t[0:1, B32_L["btrans"]:B32_L["btrans"] + 512],
                              512, "btrans_bc")
        brgcn_bc = bcast_row(b32_t[0:1, B32_L["brgcn"]:B32_L["brgcn"] + 512],
                             512, "brgcn_bc")

        nodes_g = [T(pp, [g, NF], BF16, f"nodes{gi}")
                   for gi, g in enumerate(NODE_GROUPS)]
        for gi, g in enumerate(NODE_GROUPS):
            nc.vector.tensor_copy(nodes_g[gi][:, EMB:NF],
                                  c0s(f"typ{gi}", 0, g, 20))

        wtrans = [c0s("wtrans", 0, 128, 3072)[:, k * 512:(k + 1) * 512]
                  for k in range(6)]

        # ---- S2: mention embeddings + entity logsumexp ----
        ps_memb = T(pps, [E * M, EMB], FP32, "ps")
        for kt in range(6):
            nc.tensor.matmul(ps_memb[:],
                             c0s("xg", 0, 128, 396)[:, kt * 66:(kt + 1) * 66],
                             wtrans[kt], start=(kt == 0), stop=(kt == 5))
        memb = T(pp, [E * M, EMB], FP32, "memb")
        nc.vector.tensor_add(memb[:], ps_memb[:], btrans_bc[0:E * M, :])
        nc.vector.tensor_copy(nodes_g[1][:, 0:EMB], memb[:])
        ememb = T(pp, [E * M, EMB], BF16, "ememb")
        nc.scalar.activation(ememb[:], memb[:], ACT.Exp)
        ps_ent = T(pps, [E, EMB], FP32, "ps")
        nc.tensor.matmul(ps_ent[:], c0s("g3", 0, E * M, 22), ememb[:],
                         start=True, stop=True)
        nc.scalar.activation(nodes_g[0][:, 0:EMB], ps_ent[:], ACT.Ln)

        # ---- S3: link nodes ----
        aT, aTb = [], []
        for i in range(SPAN_TILES):
            al = c0s("attl", 0, 128, SPAN_TILES * 192)[:, i * 192:(i + 1) * 192]
            a = T(pp, [128, 1], FP32, f"aT{i}")
            nc.vector.tensor_reduce(a[:], al, mybir.AxisListType.X,
                                    mybir.AluOpType.add)
            nc.vector.tensor_scalar_mul(a[:], a[:], 1.0 / 192.0)
            aT.append(a)
            ab = T(pp, [128, 1], BF16, f"aTb{i}")
            nc.vector.tensor_copy(ab[:], a[:])
            aTb.append(ab)
        gspan = [c0s("gspan", 0, 128, SPAN_TILES * 30)[:, i * 30:(i + 1) * 30]
                 for i in range(SPAN_TILES)]
        ps_as = T(pps, [L, 1], FP32, "ps")
        for i in range(SPAN_TILES):
            nc.tensor.matmul(ps_as[:], gspan[i], aTb[i][:],
                             start=(i == 0), stop=(i == SPAN_TILES - 1))
        asum = T(pp, [L, 1], FP32, "asum")
        nc.vector.tensor_copy(asum[:], ps_as[:])
        for i in range(SPAN_TILES):
            nc.vector.tensor_scalar_mul(
                c1_t[:, i * 768:(i + 1) * 768],
                c1_t[:, i * 768:(i + 1) * 768], aT[i][:])
        lct = []
        for mt in range(6):
            ps = T(pps, [128, L], FP32, "ps")
            for i in range(SPAN_TILES):
                nc.tensor.matmul(
                    ps[:], c1_t[:, i * 768 + mt * 128:i * 768 + mt * 128 + 128],
                    gspan[i], start=(i == 0), stop=(i == SPAN_TILES - 1))
            t = T(pp, [128, L], BF16, f"lct{mt}")
            nc.vector.tensor_copy(t[:], ps[:])
            lct.append(t)
        bterm = T(pp, [L, EMB], FP32, "bterm")
        nc.vector.tensor_scalar_mul(bterm[:], btrans_bc[0:L, :], asum[:])
        ps_link = T(pps, [L, EMB], FP32, "ps")
        for kt in range(6):
            nc.tensor.matmul(ps_link[:], lct[kt][:], wtrans[kt],
                             start=(kt == 0), stop=(kt == 5))
        nc.vector.tensor_add(nodes_g[2][:, 0:EMB], ps_link[:], bterm[:])

        # ---- S4: ea, then e_ctx = rcp * ((ea @ X) @ W) + b ----
        ps_ea = [T(pps, [E, 512], FP32, "ps") for _ in range(2)]
        for i in range(ATTM_TILES):
            at = c2_t[i // 4][:, (i % 4) * 1024:(i % 4) * 1024 + 1024]
            gt = c0s("gmat", 0, 128, GM_TILES * 22)[:, i * 22:(i + 1) * 22]
            for half in range(2):
                nc.tensor.matmul(ps_ea[half][:], gt,
                                 at[:, half * 512:(half + 1) * 512],
                                 start=(i == 0), stop=(i == ATTM_TILES - 1))
        ea = T(pp, [E, C], FP32, "ea")
        for half in range(2):
            nc.vector.tensor_copy(ea[:, half * 512:(half + 1) * 512],
                                  ps_ea[half][:])
        rsum = T(pp, [E, 1], FP32, "rsum")
        nc.vector.tensor_reduce(rsum[:], ea[:], mybir.AxisListType.X,
                                mybir.AluOpType.add)
        nc.vector.tensor_scalar_add(rsum[:], rsum[:], 1e-5)
        rcp = T(pp, [E, 1], FP32, "rcp")
        nc.vector.reciprocal(rcp[:], rsum[:])
        eaT = []
        for kt in range(8):
            ps = T(pps, [128, E], FP32, "ps")
            nc.tensor.transpose(ps[:], ea[:, kt * 128:(kt + 1) * 128],
                                ident[0:E, 0:E])
            t = T(pp, [128, E], BF16, f"eaT{kt}")
            nc.vector.tensor_copy(t[:], ps[:])
            eaT.append(t)
        uT = []
        for mt in range(6):
            ps = T(pps, [128, E], FP32, "ps")
            for kt in range(8):
                nc.tensor.matmul(
                    ps[:], c3_t[:, kt * 768 + mt * 128:kt * 768 + mt * 128 + 128],
                    eaT[kt][:], start=(kt == 0), stop=(kt == 7))
            t = T(pp, [128, E], BF16, f"uT{mt}")
            nc.vector.tensor_copy(t[:], ps[:])
            uT.append(t)
        ps_ectx = T(pps, [E, EMB], FP32, "ps")
        for kt in range(6):
            nc.tensor.matmul(ps_ectx[:], uT[kt][:], wtrans[kt],
                             start=(kt == 0), stop=(kt == 5))
        ectx = T(pp, [E, EMB], FP32, "ectx")
        nc.vector.tensor_scalar_mul(ectx[:], ps_ectx[:], rcp[:])
        nc.vector.tensor_add(ectx[:], ectx[:], btrans_bc[0:E, :])
        ectxT = []
        for mt in range(4):
            ps = T(pps, [128, E], FP32, "ps")
            nc.tensor.transpose(ps[:], ectx[:, mt * 128:(mt + 1) * 128],
                                ident[0:E, 0:E])
            t = T(pp, [128, E], FP32, f"ectxT{mt}")
            nc.vector.tensor_copy(t[:], ps[:])
            ectxT.append(t)

        # ---- S5: RGCN (entity outputs only) ----
        ps_cs = T(pps, [1, 88], FP32, "ps")
        for gi, g in enumerate(NODE_GROUPS):
            nc.tensor.matmul(ps_cs[:], c0s("ones16", 0, g, 1),
                             c0s(f"adjrel{gi}", 0, g, 110)[:, 0:88],
                             start=(gi == 0), stop=(gi == 2))
        cs = T(pp, [1, 88], FP32, "cs")
        nc.vector.tensor_scalar_add(cs[:], ps_cs[:], 1e-5)
        csr = T(pp, [1, 88], FP32, "csr")
        nc.vector.reciprocal(csr[:], cs[:])
        ps_csb = T(pps, [128, 88], FP32, "ps")
        nc.tensor.matmul(ps_csb[:], onesrow, csr[:], start=True, stop=True)
        csb = T(pp, [128, 88], FP32, "csb")
        nc.vector.tensor_copy(csb[:], ps_csb[:])
        adjn_g = []
        for gi, g in enumerate(NODE_GROUPS):
            t = T(pp, [g, 110], BF16, f"adjn{gi}")
            nc.vector.tensor_mul(t[:, 0:88],
                                 c0s(f"adjrel{gi}", 0, g, 110)[:, 0:88],
                                 csb[0:g, :])
            nc.vector.tensor_copy(t[:, 88:110],
                                  c0s(f"adjrel{gi}", 0, g, 110)[:, 88:110])
            adjn_g.append(t)
        NF_TILES = [(0, 128), (128, 128), (256, 128), (384, 128), (512, 20)]
        msgT = []
        for i, (off, sz) in enumerate(NF_TILES):
            ps = T(pps, [sz, 110], FP32, "ps")
            for gi in range(3):
                nc.tensor.matmul(ps[:], nodes_g[gi][:, off:off + sz],
                                 adjn_g[gi][:], start=(gi == 0), stop=(gi == 2))
            t = T(pp, [sz, 110], BF16, f"msgT{i}")
            nc.vector.tensor_copy(t[:], ps[:])
            msgT.append(t)
        ps_gcn = T(pps, [E, EMB], FP32, "ps")
        for r in range(5):
            for i, (off, sz) in enumerate(NF_TILES):
                bidx = r * 5 + i
                wv = wrel_t[bidx // 13][0:sz,
                                       (bidx % 13) * 512:(bidx % 13) * 512 + 512]
                nc.tensor.matmul(
                    ps_gcn[:], msgT[i][:, r * 22:(r + 1) * 22], wv,
                    start=(r == 0 and i == 0), stop=(r == 4 and i == 4))
        ent = T(pp, [E, EMB], FP32, "ent")
        nc.vector.tensor_add(ent[:], ps_gcn[:], brgcn_bc[0:E, :])
        nc.scalar.activation(ent[:], ent[:], ACT.Relu)
        ent16 = T(pp, [E, EMB], BF16, "ent16")
        nc.vector.tensor_copy(ent16[:], ent[:])
        entT = []
        for mt in range(4):
            ps = T(pps, [128, E], FP32, "ps")
            nc.tensor.transpose(ps[:], ent[:, mt * 128:(mt + 1) * 128],
                                ident[0:E, 0:E])
            t = T(pp, [128, E], FP32, f"entT{mt}")
            nc.vector.tensor_copy(t[:], ps[:])
            entT.append(t)

        # ---- S6: x maps + conv stack (local rows only) ----
        xpad = []
        for mt in range(4):
            xp = T(pp, [128, (NX + 4) * PW], BF16, f"xpad{mt}")
            nc.vector.memset(xp[:], 0.0)
            t1 = T(pst, [128, NX * S], FP32, "xtmp", bufs=2)
            nc.vector.tensor_mul(
                t1[:].rearrange("p (a b) -> p a b", a=NX, b=S),
                entT[mt][:, 0:NX].unsqueeze(2).to_broadcast((128, NX, S)),
                entT[mt][:].unsqueeze(1).to_broadcast((128, NX, S)))
            t2 = T(pst, [128, NX * S], FP32, "xtmp", bufs=2)
            nc.vector.tensor_mul(
                t2[:].rearrange("p (a b) -> p a b", a=NX, b=S),
                ectxT[mt][:, 0:NX].unsqueeze(2).to_broadcast((128, NX, S)),
                ectxT[mt][:].unsqueeze(1).to_broadcast((128, NX, S)))
            inner = xp[:].rearrange("p (a b) -> p a b", a=NX + 4, b=PW)[
                :, 2:2 + NX, 2:2 + S]
            nc.vector.tensor_add(inner, t1[:], t2[:])
            xpad.append(xp)

        def conv(in_tiles, in_rows, wsel, oc, n_out, out_cb):
            """kt-outer 5x5 conv so weight chunks are consumed sequentially."""
            n_ic_t, n_oc_t = len(in_tiles), oc // 128
            ps_c = [T(pps, [128, n_out * S], FP32, "ps") for _ in range(n_oc_t)]
            n_acc = 25 * n_ic_t
            a = 0
            for kt in range(n_ic_t):
                for tap in range(25):
                    di, dj = divmod(tap, 5)
                    rhs = in_tiles[kt][:].rearrange(
                        "p (a b) -> p a b", a=in_rows + 4, b=PW)[
                        :, di:di + n_out, dj:dj + S]
                    w = wsel(kt, tap)
                    for mt in range(n_oc_t):
                        nc.tensor.matmul(
                            ps_c[mt][:], w[:, mt * 128:mt * 128 + 128],
                            rhs, start=(a == 0), stop=(a == n_acc - 1))
                    a += 1
            for mt in range(n_oc_t):
                out_cb(mt, ps_c[mt])

        pad1 = []
        for mt in range(2):
            t = T(pp, [128, (N1 + 4) * PW], BF16, f"pad1_{mt}")
            nc.vector.memset(t[:], 0.0)
            pad1.append(t)

        def c1_out(mt, ps):
            inner = pad1[mt][:].rearrange("p (a b) -> p a b",
                                          a=N1 + 4, b=PW)[:, 2:2 + N1, 2:2 + S]
            nc.scalar.activation(
                inner, ps[:].rearrange("p (a b) -> p a b", a=N1, b=S),
                ACT.Relu, bias=b32_t[:, B32_L["b1"] + mt:B32_L["b1"] + mt + 1])

        conv(xpad, NX, lambda kt, tap: w1_t[kt][:, tap * IC:tap * IC + IC],
             IC, N1, c1_out)

        pad2 = []
        for mt in range(2):
            t = T(pp, [128, (N2 + 4) * PW], BF16, f"pad2_{mt}")
            nc.vector.memset(t[:], 0.0)
            pad2.append(t)

        def c2_out(mt, ps):
            inner = pad2[mt][:].rearrange("p (a b) -> p a b",
                                          a=N2 + 4, b=PW)[:, 2:2 + N2, 2:2 + S]
            nc.scalar.activation(
                inner, ps[:].rearrange("p (a b) -> p a b", a=N2, b=S),
                ACT.Relu, bias=b32_t[:, B32_L["b2"] + mt:B32_L["b2"] + mt + 1])

        conv(pad1, N1, lambda kt, tap: w2_t[kt][:, tap * IC:tap * IC + IC],
             IC, N2, c2_out)

        x3 = [T(pp, [128, SP3], BF16, f"x3_{mt}") for mt in range(4)]

        def c3_out(mt, ps):
            nc.scalar.activation(
                x3[mt][:], ps[:], ACT.Relu,
                bias=b32_t[:, B32_L["b3"] + mt:B32_L["b3"] + mt + 1])

        def w3sel(kt, tap):
            half = tap // 13
            lo = (tap - 13 * half) * EMB
            return w3_t[kt * 2 + half][:, lo:lo + EMB]

        conv(pad2, N2, w3sel, EMB, N3, c3_out)

        # ---- S7: pair features + classifier ----
        x3T = []
        for i, (off, sz) in enumerate(SP3_TILES):
            t = T(pp, [sz, EMB], BF16, f"x3T{i}")
            x3T.append(t)
            for src in range(4):
                ps = T(pps, [sz, 64], FP32, "ps")
                psb = ps[:].bitcast(BF16)
                nc.tensor.transpose(psb, x3[src][:, off:off + sz],
                                    identp_t[:, :])
                nc.vector.tensor_copy(t[:, src * 128:(src + 1) * 128], psb)

        sh = c4_t[0:E, C4_L["sh"]:C4_L["sh"] + PH2]
        st = c4_t[0:E, C4_L["st"]:C4_L["st"] + PH2]
        sm = [c4_t[:, C4_L["sm"] + i * PH2:C4_L["sm"] + (i + 1) * PH2]
              for i in range(2)]

        featT = [None] * 16
        for mt in range(4):
            ps = T(pps, [128, PH2], FP32, "ps")
            nc.tensor.matmul(ps[:], ent16[:, mt * 128:(mt + 1) * 128], sh,
                             start=True, stop=True)
            t = T(pp, [128, PH2], BF16, f"featT{mt}")
            nc.vector.tensor_copy(t[:], ps[:])
            featT[mt] = t
        for mt in range(4):
            ps = T(pps, [128, PH2], FP32, "ps")
            nc.tensor.matmul(ps[:], ent16[:, mt * 128:(mt + 1) * 128], st,
                             start=True, stop=True)
            t = T(pp, [128, PH2], BF16, f"featT{4 + mt}")
            nc.vector.tensor_copy(t[:], ps[:])
            featT[4 + mt] = t
        for mt in range(4):
            ps = T(pps, [128, PH2], FP32, "ps")
            for i, (off, sz) in enumerate(SP3_TILES):
                nc.tensor.matmul(ps[:], x3T[i][:, mt * 128:(mt + 1) * 128],
                                 sm[i][0:sz, :], start=(i == 0), stop=(i == 1))
            t = T(pp, [128, PH2], BF16, f"featT{8 + mt}")
            nc.vector.tensor_copy(t[:], ps[:])
            featT[8 + mt] = t
        for mt in range(4):
            t = T(pp, [128, PH2], BF16, f"featT{12 + mt}")
            nc.vector.tensor_mul(t[:], featT[mt][:], featT[4 + mt][:])
            featT[12 + mt] = t

        ps_ht = [T(pps, [128, PH2], FP32, "ps") for _ in range(8)]
        for kt in range(16):
            w = wht_t[kt // 4]
            for mt in range(8):
                nc.tensor.matmul(
                    ps_ht[mt][:],
                    w[:, (kt % 4) * 1024 + mt * 128:(kt % 4) * 1024 + mt * 128 + 128],
                    featT[kt][:], start=(kt == 0), stop=(kt == 15))
        htT = []
        for mt in range(8):
            t = T(pp, [128, PH2], BF16, f"htT{mt}")
            nc.scalar.activation(
                t[:], ps_ht[mt][:], ACT.Tanh,
                bias=b32_t[:, B32_L["bht"] + mt:B32_L["bht"] + mt + 1])
            htT.append(t)

        ps_out = T(pps, [97, PH2], FP32, "ps")
        for kt in range(8):
            nc.tensor.matmul(ps_out[:], wbil_t[:, kt * 97:(kt + 1) * 97],
                             htT[kt][:], start=(kt == 0), stop=(kt == 7))
        out_sb = T(pp, [97, PH2], FP32, "out")
        nc.vector.tensor_scalar_add(
            out_sb[:], ps_out[:],
            b32_t[0:97, B32_L["bbil"]:B32_L["bbil"] + 1])
        dma(outt, out_sb[:])

    nc.compile()
    return nc


_PROG = None


def _get_prog():
    global _PROG
    if _PROG is None:
        _PROG = build_program()
    return _PROG


def _padtiles(arr, tile_rows=128):
    """[R, W] -> [128, ceil(R/128)*W] column-blocked, rows zero-padded."""
    r, w = arr.shape
    nt = (r + tile_rows - 1) // tile_rows
    out = np.zeros((tile_rows, nt * w), arr.dtype)
    for i in range(nt):
        blk = arr[i * tile_rows:(i + 1) * tile_rows]
        out[:blk.shape[0], i * w:i * w + w] = blk
    return out


def _shared_inputs(inputs):
    f32 = np.float32
    bf = ml_dtypes.bfloat16
    sh = {}
    sh["identp"] = np.eye(128, dtype=bf)
    # blob32
    b32 = np.zeros((128, B32_W), f32)
    b32[:, B32_L["ident"]:B32_L["ident"] + 128] = np.eye(128, dtype=f32)
    b32[:, B32_L["ones"]] = 1.0
    b32[0, B32_L["onesrow"]:B32_L["onesrow"] + 128] = 1.0
    b32[0, B32_L["btrans"]:B32_L["btrans"] + 512] = np.asarray(
        inputs["b_trans"], f32)
    b32[0, B32_L["brgcn"]:B32_L["brgcn"] + 512] = np.asarray(
        inputs["b_rgcn"], f32)
    b32[:, B32_L["b1"]:B32_L["b1"] + 2] = np.asarray(
        inputs["conv1_b"], f32).reshape(2, 128).T
    b32[:, B32_L["b2"]:B32_L["b2"] + 2] = np.asarray(
        inputs["conv2_b"], f32).reshape(2, 128).T
    b32[:, B32_L["b3"]:B32_L["b3"] + 4] = np.asarray(
        inputs["conv3_b"], f32).reshape(4, 128).T
    b32[:, B32_L["bht"]:B32_L["bht"] + 8] = np.asarray(
        inputs["ht_b"], f32).reshape(8, 128).T
    b32[0:97, B32_L["bbil"]] = np.asarray(inputs["bil_b"], f32)
    sh["b32"] = b32
    # wrel: 25 blocks (r*5+i) of [<=128, 512], 13 blocks per WCH chunk
    wcat = np.concatenate(
        [np.asarray(inputs["W_rel"], f32).reshape(4 * NF, EMB),
         np.asarray(inputs["W_self"], f32)], axis=0)
    wr = np.zeros((128, 2 * WCH), f32)
    for r in range(5):
        for i in range(5):
            blk = wcat[r * NF + i * 128:r * NF + min((i + 1) * 128, NF)]
            bidx = r * 5 + i
            col = (bidx // 13) * WCH + (bidx % 13) * 512
            wr[:blk.shape[0], col:col + 512] = blk
    sh["wrel"] = wr.astype(bf)
    # conv weights, two spatial variants
    def convpack(w, nkt, oc, nchunk):
        # w [oc, ic, 5, 5] -> [128, nchunk*WCH]; within a chunk, col tap*oc + o
        t = np.ascontiguousarray(np.asarray(w, f32).transpose(1, 2, 3, 0))
        t = t.reshape(nkt, 128, 25, oc).transpose(1, 0, 2, 3)  # [128,kt,tap,oc]
        out = np.zeros((128, nchunk * WCH), f32)
        taps_per = 25 // (nchunk // nkt) if nchunk != nkt else 25
        if nchunk == nkt:           # whole kt fits one chunk (25*oc <= WCH)
            for kt in range(nkt):
                out[:, kt * WCH:kt * WCH + 25 * oc] = t[:, kt].reshape(128, -1)
        else:                       # kt split into 2 chunks: taps 0-12 / 13-24
            for kt in range(nkt):
                for half, (t0, t1) in enumerate([(0, 13), (13, 25)]):
                    ch = kt * 2 + half
                    blk = t[:, kt, t0:t1].reshape(128, -1)
                    out[:, ch * WCH:ch * WCH + blk.shape[1]] = blk
        return np.ascontiguousarray(out).astype(bf)
    for hh in range(2):
        w1 = np.asarray(inputs["conv1_w"], f32)
        w2 = np.asarray(inputs["conv2_w"], f32)
        w3 = np.asarray(inputs["conv3_w"], f32)
        if hh == 1:
            w1 = w1[:, :, ::-1, ::-1]
            w2 = w2[:, :, ::-1, ::-1]
            w3 = w3[:, :, ::-1, ::-1]
        sh[f"w1_{hh}"] = convpack(w1, 4, IC, 4)
        sh[f"w2_{hh}"] = convpack(w2, 2, IC, 2)
        sh[f"w3_{hh}"] = convpack(w3, 2, EMB, 4)
    sh["wht"] = np.ascontiguousarray(
        np.asarray(inputs["ht_W"], f32).reshape(16, 128, 1024)
        .transpose(1, 0, 2).reshape(128, 16 * 1024)).astype(bf)
    sh["wbil"] = np.ascontiguousarray(
        np.asarray(inputs["bil_W"], f32).reshape(8, 128, 97)
        .transpose(1, 0, 2).reshape(128, 8 * 97)).astype(bf)
    # structural constants
    sh["_g3"] = np.kron(np.eye(E, dtype=f32), np.ones((M, 1), f32))
    sh["_gmat"] = np.kron(np.eye(E, dtype=f32),
                          np.ones((M * NH, 1), f32) / (M * NH))
    sh["_gspan"] = np.kron(np.eye(L, dtype=f32), np.ones((LS, 1), f32))
    sh["_wtrans"] = np.asarray(inputs["W_trans"], f32)
    sh["_type_embed"] = np.asarray(inputs["type_embed"], f32)
    return sh


def _core_inputs(inputs, shared, b, hh):
    f32 = np.float32
    bf = ml_dtypes.bfloat16
    if hh == 0:
        perm_e = np.arange(E)
    else:
        perm_e = np.arange(E)[::-1]
    perm_m = (perm_e[:, None] * M + np.arange(M)).reshape(-1)
    p_nodes = np.concatenate([perm_e, E + perm_m, np.arange(E + E * M, NN)])
    X = np.asarray(inputs["sequence_output"][b], f32)
    att = np.asarray(inputs["attention"][b], f32)
    adj = np.asarray(inputs["adjacency"][b], f32)[:, p_nodes][:, :, p_nodes]
    mf = np.asarray(inputs["mention_idx"][b]).astype(np.int64)[perm_e].reshape(-1)
    ls = np.asarray(inputs["link_start"][b]).astype(np.int64)
    ntypes = np.asarray(inputs["node_types"][b]).astype(np.int64)[p_nodes]
    hts = np.asarray(inputs["hts"][b]).astype(np.int64)

    # C0
    c0 = np.zeros((128, C0_W), bf)
    adjrel = np.zeros((NN, 110), f32)
    for r in range(4):
        adjrel[:, r * 22:(r + 1) * 22] = adj[r].T[:, :E]
    adjrel[:, 88:110] = np.eye(NN, dtype=f32)[:, :E]
    typ_all = shared["_type_embed"][ntypes]
    goff = 0
    for gi, g in enumerate([22, 66, 30]):
        c0[0:g, C0_L[f"adjrel{gi}"]:C0_L[f"adjrel{gi}"] + 110] = \
            adjrel[goff:goff + g].astype(bf)
        c0[0:g, C0_L[f"typ{gi}"]:C0_L[f"typ{gi}"] + 20] = \
            typ_all[goff:goff + g].astype(bf)
        goff += g
    c0[0:E * M, C0_L["g3"]:C0_L["g3"] + 22] = shared["_g3"].astype(bf)
    c0[:, C0_L["ones16"]] = 1.0
    c0[:, C0_L["gmat"]:C0_L["gmat"] + GM_TILES * 22] = _padtiles(
        shared["_gmat"].astype(bf))
    c0[:, C0_L["gspan"]:C0_L["gspan"] + SPAN_TILES * 30] = _padtiles(
        shared["_gspan"].astype(bf))
    pos = ls[:, None] + np.arange(LS)
    attl = np.empty((L * LS, NH * LS), f32)
    for l in range(L):
        blk = att[:, pos[l]][:, :, pos[l]]
        attl[l * LS:(l + 1) * LS] = blk.transpose(2, 0, 1).reshape(LS, NH * LS)
    c0[:, C0_L["attl"]:C0_L["attl"] + SPAN_TILES * 192] = _padtiles(
        attl.astype(bf))
    c0[:, C0_L["xg"]:C0_L["xg"] + 6 * 66] = _padtiles(
        np.ascontiguousarray(X[mf].T).astype(bf))
    c0[:, C0_L["wtrans"]:C0_L["wtrans"] + 6 * 512] = _padtiles(
        shared["_wtrans"].astype(bf))
    # C1 xspan, C2 attm, C3 xn
    c1 = _padtiles(X[pos.reshape(-1)].astype(bf))
    attm = np.ascontiguousarray(
        att[:, mf].transpose(1, 0, 2).reshape(E * M * NH, C))
    c2 = np.zeros((128, 8192), bf)
    c2[:, :C2_W] = _padtiles(attm.astype(bf))
    c3 = _padtiles(X.astype(bf))
    # C4 selectors (local band: h' in 0..10)
    inv = np.empty(E, np.int64)
    inv[perm_e] = np.arange(E)
    hl, tl = inv[hts[:, 0]], inv[hts[:, 1]]
    sel = np.where(hl <= 10)[0]
    assert len(sel) <= PH2, f"band overflow: {len(sel)} > {PH2}"
    c4 = np.zeros((128, C4_W), bf)
    shm = np.zeros((E, PH2), f32)
    shm[hl[sel], np.arange(len(sel))] = 1.0
    stm = np.zeros((E, PH2), f32)
    stm[tl[sel], np.arange(len(sel))] = 1.0
    smm = np.zeros((SP3, PH2), f32)
    smm[hl[sel] * S + tl[sel], np.arange(len(sel))] = 1.0
    c4[0:E, C4_L["sh"]:C4_L["sh"] + PH2] = shm.astype(bf)
    c4[0:E, C4_L["st"]:C4_L["st"] + PH2] = stm.astype(bf)
    c4[:, C4_L["sm"]:C4_L["sm"] + 2 * PH2] = _padtiles(smm.astype(bf))

    m = {
        "identp": shared["identp"], "b32": shared["b32"],
        "wrel": shared["wrel"], "wht": shared["wht"], "wbil": shared["wbil"],
        "w1": shared[f"w1_{hh}"], "w2": shared[f"w2_{hh}"],
        "w3": shared[f"w3_{hh}"],
        "c0": c0, "c1": c1, "c2": c2, "c3": c3, "c4": c4,
    }
    return m, sel


def kernel(**inputs):
    nc = _get_prog()
    shared = _shared_inputs(inputs)
    in_maps, sels = [], []
    for b in range(B):
        for hh in range(2):
            m, sel = _core_inputs(inputs, shared, b, hh)
            in_maps.append(m)
            sels.append(sel)
    res = run_bass_kernel_spmd(nc, in_maps, core_ids=list(range(8)))
    out = np.empty((B, P, 97), np.float32)
    for b in range(B):
        for hh in range(2):
            ci = 2 * b + hh
            sel = sels[ci]
            r = np.asarray(res.results[ci]["outt"], np.float32)
            out[b, sel, :] = r[:, :len(sel)].T
    return out


# revision 4
# speedup vs baseline: 1.7066x; 1.0814x over previous
"""Trainium2 Bass kernel for nn_DocREModel (8-core SPMD), v2.

Sharding: 4 docs x 2 half-bands = 8 cores. Each core runs an identical
program. The pair of cores sharing a doc splits the conv stack spatially:
each computes conv3 output rows 0..10 in ITS OWN entity coordinates.
Odd cores see the document with entity order reflected (e -> 21-e) and
spatially-flipped conv kernels, which makes "rows 0..10 local" equal to
global rows 11..21 — so both halves run the same compiled program.

Host does only index-driven data movement: batch slicing, transposes,
row gathers at integer indices, one-hot/selector construction, dtype
casts, and packing into wide-row DMA blobs. All FP arithmetic runs on
device.
"""

import numpy as np
from contextlib import ExitStack

import concourse.bass as bass
import concourse.bacc as bacc
import concourse.tile as tile
import concourse.mybir as mybir
from concourse.bass_utils import run_bass_kernel_spmd

import ml_dtypes

FP32 = mybir.dt.float32
BF16 = mybir.dt.bfloat16
_NPDT = {FP32: np.float32, BF16: ml_dtypes.bfloat16}

B, C, H, NH = 4, 1024, 768, 12
E, M, L, LS = 22, 3, 30, 16
NN, NF, EMB = 118, 532, 512
P, PH2, IC = 462, 256, 256
S = 22
ACT = mybir.ActivationFunctionType

# conv split geometry (local coords; identical on every core)
NX, N1, N2, N3 = 17, 15, 13, 11      # x rows 0..16 -> c1 0..14 -> c2 0..12 -> c3 0..10
PW = 26                               # padded cols (-2..23)
SP3 = N3 * S                          # 242 spatial positions of x3
SP3_TILES = [(0, 128), (128, SP3 - 128)]   # 128 + 114

ATTM_TILES = 7                        # 792 rows -> 7x128 (pad)
SPAN_TILES = 4                        # 480 rows -> 4x128 (pad)
GM_TILES = 7

# ---- C0..C4 blob16 layouts: (name, rows, cols[, count]) ----
def _layout(entries):
    off, d = 0, {}
    for name, cols in entries:
        d[name] = off
        off += cols
    return d, off

NODE_GROUPS = [22, 66, 30]            # entities, mentions, links

C0_L, C0_W = _layout([
    ("adjrel0", 110), ("adjrel1", 110), ("adjrel2", 110),
    ("typ0", 20), ("typ1", 20), ("typ2", 20),
    ("g3", 22), ("ones16", 1), ("gmat", GM_TILES * 22),
    ("gspan", SPAN_TILES * 30), ("attl", SPAN_TILES * 192),
    ("xg", 6 * 66), ("wtrans", 6 * 512),
])
C4_L, C4_W = _layout([("sh", PH2), ("st", PH2), ("sm", 2 * PH2)])
C1_W = 4 * 768        # xspan
C2_W = 7 * 1024       # attm
C3_W = 8 * 768        # xn

WCH = 13 * 512            # unified weight-chunk width (6656 cols bf16)

# blob32 layout
B32_L, B32_W = _layout([
    ("ident", 128), ("ones", 1), ("onesrow", 128), ("btrans", 512),
    ("brgcn", 512), ("b1", 2), ("b2", 2), ("b3", 4), ("bht", 8), ("bbil", 1),
])


def build_program():
    nc = bacc.Bacc("TRN2", target_bir_lowering=False, debug=False)

    def din(name, shape, dt=BF16):
        return nc.dram_tensor(name, shape, dt, kind="ExternalInput").ap()

    identp_d = din("identp", [128, 128])
    c0_d = din("c0", [128, C0_W])
    c1_d = din("c1", [128, C1_W])
    c2_d = din("c2", [128, 8192])
    c3_d = din("c3", [128, C3_W])
    c4_d = din("c4", [128, C4_W])
    b32_d = din("b32", [128, B32_W], FP32)
    wrel_d = din("wrel", [128, 2 * WCH])
    w1_d = din("w1", [128, 4 * WCH])
    w2_d = din("w2", [128, 2 * WCH])
    w3_d = din("w3", [128, 4 * WCH])
    wht_d = din("wht", [128, 16 * 1024])
    wbil_d = din("wbil", [128, 8 * 97])
    outt = nc.dram_tensor("outt", [97, PH2], FP32, kind="ExternalOutput").ap()

    with tile.TileContext(nc) as tc, ExitStack() as ctx:
        pp = ctx.enter_context(tc.tile_pool(name="persist", bufs=1))
        pst = ctx.enter_context(tc.tile_pool(name="stream", bufs=1))
        pps = ctx.enter_context(tc.tile_pool(name="psum", bufs=8, space="PSUM"))
        pdram = ctx.enter_context(tc.tile_pool(name="dram", bufs=1, space="DRAM"))

        dma = nc.sync.dma_start
        gdma = nc.gpsimd.dma_start

        def T(pool, shape, dt, tag, bufs=None):
            return pool.tile(shape, dt, tag=tag, name=tag, bufs=bufs)

        # ---- DMAs (order within each ring matters) ----
        # ring order = priority: graph-phase inputs first on BOTH rings,
        # weight streams after.
        identp_t = T(pp, [128, 128], BF16, "identp")
        dma(identp_t[:], identp_d)
        # gpsimd ring: c1, c3, wrel, w1, w2
        c1_t = T(pp, [128, C1_W], BF16, "c1")
        gdma(c1_t[:], c1_d)
        c3_t = T(pp, [128, C3_W], BF16, "c3")
        gdma(c3_t[:], c3_d)
        # sync ring: identp, b32, c0, c2, c4, w3, wht, wbil
        b32_t = T(pp, [128, B32_W], FP32, "b32")
        dma(b32_t[:], b32_d)
        c0_t = T(pp, [128, C0_W], BF16, "c0")
        dma(c0_t[:], c0_d)
        # attm (2 chunks) shares a [128, 4096] ring with wht (4 chunks)
        c2_t = []
        for k in range(2):
            t = T(pst, [128, 4096], BF16, "b4k", bufs=2)
            dma(t[:], c2_d[:, k * 4096:(k + 1) * 4096])
            c2_t.append(t)
        c4_t = T(pp, [128, C4_W], BF16, "c4")
        dma(c4_t[:], c4_d)
        # unified weight-chunk ring: wrel(2), w1(4), w2(2), w3(4)
        def wchunk(dram, k, engine_dma):
            t = T(pst, [128, WCH], BF16, "wchunk", bufs=4)
            engine_dma(t[:], dram[:, k * WCH:(k + 1) * WCH])
            return t

        wrel_t = [wchunk(wrel_d, k, gdma) for k in range(2)]
        w1_t = [wchunk(w1_d, k, gdma) for k in range(4)]
        w2_t = [wchunk(w2_d, k, gdma) for k in range(2)]
        w3_t = [wchunk(w3_d, k, dma) for k in range(4)]
        wht_t = []
        for k in range(4):
            t = T(pst, [128, 4096], BF16, "b4k", bufs=2)
            dma(t[:], wht_d[:, k * 4096:(k + 1) * 4096])
            wht_t.append(t)
        wbil_t = T(pp, [128, 8 * 97], BF16, "wbil")
        dma(wbil_t[:], wbil_d)

        def c0s(name, r0, r1, w):
            return c0_t[r0:r1, C0_L[name]:C0_L[name] + w]

        ident = b32_t[:, B32_L["ident"]:B32_L["ident"] + 128]
        onesrow = b32_t[0:1, B32_L["onesrow"]:B32_L["onesrow"] + 128]

        # ---- warmup matmuls: flip PE HAM to 8/8 during initial DMA wait ----
        ps_warm = T(pps, [128, 128], FP32, "ps")
        for _ in range(72):
            nc.tensor.matmul(ps_warm[:], identp_t[:], identp_t[:],
                             start=True, stop=True)
        warm_sb = T(pp, [128, 128], FP32, "warm_sb")
        nc.vector.tensor_copy(warm_sb[:], ps_warm[:])
        warm_dram = pdram.tile([128, 128], FP32, name="warm_dram")
        dma(warm_dram[:], warm_sb[:])

        # ---- broadcast rows via K=1 matmuls ----
        def bcast_row(src_row_ap, w, tag):
            ps = T(pps, [128, w], FP32, "ps")
            nc.tensor.matmul(ps[:], onesrow, src_row_ap, start=True, stop=True)
            t = T(pp, [128, w], FP32, tag)
            nc.vector.tensor_copy(t[:], ps[:])
            return t

        btrans_bc = bcast_row(b32_t[0:1, B32_L["btrans"]:B32_L["btrans"] + 512],
                              512, "btrans_bc")
        brgcn_bc = bcast_row(b32_t[0:1, B32_L["brgcn"]:B32_L["brgcn"] + 512],
                             512, "brgcn_bc")

        nodes_g = [T(pp, [g, NF], BF16, f"nodes{gi}")
                   for gi, g in enumerate(NODE_GROUPS)]
        for gi, g in enumerate(NODE_GROUPS):
            nc.vector.tensor_copy(nodes_g[gi][:, EMB:NF],
                                  c0s(f"typ{gi}", 0, g, 20))

        wtrans = [c0s("wtrans", 0, 128, 3072)[:, k * 512:(k + 1) * 512]
                  for k in range(6)]

        # ---- S2: mention embeddings + entity logsumexp ----
        ps_memb = T(pps, [E * M, EMB], FP32, "ps")
        for kt in range(6):
            nc.tensor.matmul(ps_memb[:],
                             c0s("xg", 0, 128, 396)[:, kt * 66:(kt + 1) * 66],
                             wtrans[kt], start=(kt == 0), stop=(kt == 5))
        memb = T(pp, [E * M, EMB], FP32, "memb")
        nc.vector.tensor_add(memb[:], ps_memb[:], btrans_bc[0:E * M, :])
        nc.vector.tensor_copy(nodes_g[1][:, 0:EMB], memb[:])
        ememb = T(pp, [E * M, EMB], BF16, "ememb")
        nc.scalar.activation(ememb[:], memb[:], ACT.Exp)
        ps_ent = T(pps, [E, EMB], FP32, "ps")
        nc.tensor.matmul(ps_ent[:], c0s("g3", 0, E * M, 22), ememb[:],
                         start=True, stop=True)
        nc.scalar.activation(nodes_g[0][:, 0:EMB], ps_ent[:], ACT.Ln)

        # ---- S3: link nodes ----
        aT, aTb = [], []
        for i in range(SPAN_TILES):
            al = c0s("attl", 0, 128, SPAN_TILES * 192)[:, i * 192:(i + 1) * 192]
            a = T(pp, [128, 1], FP32, f"aT{i}")
            nc.vector.tensor_reduce(a[:], al, mybir.AxisListType.X,
                                    mybir.AluOpType.add)
            nc.vector.tensor_scalar_mul(a[:], a[:], 1.0 / 192.0)
            aT.append(a)
            ab = T(pp, [128, 1], BF16, f"aTb{i}")
            nc.vector.tensor_copy(ab[:], a[:])
            aTb.append(ab)
        gspan = [c0s("gspan", 0, 128, SPAN_TILES * 30)[:, i * 30:(i + 1) * 30]
                 for i in range(SPAN_TILES)]
        ps_as = T(pps, [L, 1], FP32, "ps")
        for i in range(SPAN_TILES):
            nc.tensor.matmul(ps_as[:], gspan[i], aTb[i][:],
                             start=(i == 0), stop=(i == SPAN_TILES - 1))
        asum = T(pp, [L, 1], FP32, "asum")
        nc.vector.tensor_copy(asum[:], ps_as[:])
        for i in range(SPAN_TILES):
            nc.vector.tensor_scalar_mul(
                c1_t[:, i * 768:(i + 1) * 768],
                c1_t[:, i * 768:(i + 1) * 768], aT[i][:])
        lct = []
        for mt in range(6):
            ps = T(pps, [128, L], FP32, "ps")
            for i in range(SPAN_TILES):
                nc.tensor.matmul(
                    ps[:], c1_t[:, i * 768 + mt * 128:i * 768 + mt * 128 + 128],
                    gspan[i], start=(i == 0), stop=(i == SPAN_TILES - 1))
            t = T(pp, [128, L], BF16, f"lct{mt}")
            nc.vector.tensor_copy(t[:], ps[:])
            lct.append(t)
        bterm = T(pp, [L, EMB], FP32, "bterm")
        nc.vector.tensor_scalar_mul(bterm[:], btrans_bc[0:L, :], asum[:])
        ps_link = T(pps, [L, EMB], FP32, "ps")
        for kt in range(6):
            nc.tensor.matmul(ps_link[:], lct[kt][:], wtrans[kt],
                             start=(kt == 0), stop=(kt == 5))
        nc.vector.tensor_add(nodes_g[2][:, 0:EMB], ps_link[:], bterm[:])

        # ---- S4: ea, then e_ctx = rcp * ((ea @ X) @ W) + b ----
        ps_ea = [T(pps, [E, 512], FP32, "ps") for _ in range(2)]
        for i in range(ATTM_TILES):
            at = c2_t[i // 4][:, (i % 4) * 1024:(i % 4) * 1024 + 1024]
            gt = c0s("gmat", 0, 128, GM_TILES * 22)[:, i * 22:(i + 1) * 22]
            for half in range(2):
                nc.tensor.matmul(ps_ea[half][:], gt,
                                 at[:, half * 512:(half + 1) * 512],
                                 start=(i == 0), stop=(i == ATTM_TILES - 1))
        ea = T(pp, [E, C], FP32, "ea")
        for half in range(2):
            nc.vector.tensor_copy(ea[:, half * 512:(half + 1) * 512],
                                  ps_ea[half][:])
        rsum = T(pp, [E, 1], FP32, "rsum")
        nc.vector.tensor_reduce(rsum[:], ea[:], mybir.AxisListType.X,
                                mybir.AluOpType.add)
        nc.vector.tensor_scalar_add(rsum[:], rsum[:], 1e-5)
        rcp = T(pp, [E, 1], FP32, "rcp")
        nc.vector.reciprocal(rcp[:], rsum[:])
        eaT = []
        for kt in range(8):
            ps = T(pps, [128, E], FP32, "ps")
            nc.tensor.transpose(ps[:], ea[:, kt * 128:(kt + 1) * 128],
                                ident[0:E, 0:E])
            t = T(pp, [128, E], BF16, f"eaT{kt}")
            nc.vector.tensor_copy(t[:], ps[:])
            eaT.append(t)
        uT = []
        for mt in range(6):
            ps = T(pps, [128, E], FP32, "ps")
            for kt in range(8):
                nc.tensor.matmul(
                    ps[:], c3_t[:, kt * 768 + mt * 128:kt * 768 + mt * 128 + 128],
                    eaT[kt][:], start=(kt == 0), stop=(kt == 7))
            t = T(pp, [128, E], BF16, f"uT{mt}")
            nc.vector.tensor_copy(t[:], ps[:])
            uT.append(t)
        ps_ectx = T(pps, [E, EMB], FP32, "ps")
        for kt in range(6):
            nc.tensor.matmul(ps_ectx[:], uT[kt][:], wtrans[kt],
                             start=(kt == 0), stop=(kt == 5))
        ectx = T(pp, [E, EMB], FP32, "ectx")
        nc.vector.tensor_scalar_mul(ectx[:], ps_ectx[:], rcp[:])
        nc.vector.tensor_add(ectx[:], ectx[:], btrans_bc[0:E, :])
        ectxT = []
        for mt in range(4):
            ps = T(pps, [128, E], FP32, "ps")
            nc.tensor.transpose(ps[:], ectx[:, mt * 128:(mt + 1) * 128],
                                ident[0:E, 0:E])
            t = T(pp, [128, E], FP32, f"ectxT{mt}")
            nc.vector.tensor_copy(t[:], ps[:])
            ectxT.append(t)

        # ---- S5: RGCN (entity outputs only) ----
        ps_cs = T(pps, [1, 88], FP32, "ps")
        for gi, g in enumerate(NODE_GROUPS):
            nc.tensor.matmul(ps_cs[:], c0s("ones16", 0, g, 1),
                             c0s(f"adjrel{gi}", 0, g, 110)[:, 0:88],
                             start=(gi == 0), stop=(gi == 2))
        cs = T(pp, [1, 88], FP32, "cs")
        nc.vector.tensor_scalar_add(cs[:], ps_cs[:], 1e-5)
        csr = T(pp, [1, 88], FP32, "csr")
        nc.vector.reciprocal(csr[:], cs[:])
        ps_csb = T(pps, [128, 88], FP32, "ps")
        nc.tensor.matmul(ps_csb[:], onesrow, csr[:], start=True, stop=True)
        csb = T(pp, [128, 88], FP32, "csb")
        nc.vector.tensor_copy(csb[:], ps_csb[:])
        adjn_g = []
        for gi, g in enumerate(NODE_GROUPS):
            t = T(pp, [g, 110], BF16, f"adjn{gi}")
            nc.vector.tensor_mul(t[:, 0:88],
                                 c0s(f"adjrel{gi}", 0, g, 110)[:, 0:88],
                                 csb[0:g, :])
            nc.vector.tensor_copy(t[:, 88:110],
                                  c0s(f"adjrel{gi}", 0, g, 110)[:, 88:110])
            adjn_g.append(t)
        NF_TILES = [(0, 128), (128, 128), (256, 128), (384, 128), (512, 20)]
        msgT = []
        for i, (off, sz) in enumerate(NF_TILES):
            ps = T(pps, [sz, 110], FP32, "ps")
            for gi in range(3):
                nc.tensor.matmul(ps[:], nodes_g[gi][:, off:off + sz],
                                 adjn_g[gi][:], start=(gi == 0), stop=(gi == 2))
            t = T(pp, [sz, 110], BF16, f"msgT{i}")
            nc.vector.tensor_copy(t[:], ps[:])
            msgT.append(t)
        ps_gcn = T(pps, [E, EMB], FP32, "ps")
        for r in range(5):
            for i, (off, sz) in enumerate(NF_TILES):
                bidx = r * 5 + i
                wv = wrel_t[bidx // 13][0:sz,
                                       (bidx % 13) * 512:(bidx % 13) * 512 + 512]
                nc.tensor.matmul(
                    ps_gcn[:], msgT[i][:, r * 22:(r + 1) * 22], wv,
                    start=(r == 0 and i == 0), stop=(r == 4 and i == 4))
        ent = T(pp, [E, EMB], FP32, "ent")
        nc.vector.tensor_add(ent[:], ps_gcn[:], brgcn_bc[0:E, :])
        nc.scalar.activation(ent[:], ent[:], ACT.Relu)
        ent16 = T(pp, [E, EMB], BF16, "ent16")
        nc.vector.tensor_copy(ent16[:], ent[:])
        entT = []
        for mt in range(4):
            ps = T(pps, [128, E], FP32, "ps")
            nc.tensor.transpose(ps[:], ent[:, mt * 128:(mt + 1) * 128],
                                ident[0:E, 0:E])
            t = T(pp, [128, E], FP32, f"entT{mt}")
            nc.vector.tensor_copy(t[:], ps[:])
            entT.append(t)

        # ---- S6: x maps + conv stack (local rows only) ----
        xpad = []
        for mt in range(4):
            xp = T(pp, [128, (NX + 4) * PW], BF16, f"xpad{mt}")
            nc.vector.memset(xp[:], 0.0)
            t1 = T(pst, [128, NX * S], FP32, "xtmp", bufs=2)
            nc.vector.tensor_mul(
                t1[:].rearrange("p (a b) -> p a b", a=NX, b=S),
                entT[mt][:, 0:NX].unsqueeze(2).to_broadcast((128, NX, S)),
                entT[mt][:].unsqueeze(1).to_broadcast((128, NX, S)))
            t2 = T(pst, [128, NX * S], FP32, "xtmp", bufs=2)
            nc.vector.tensor_mul(
                t2[:].rearrange("p (a b) -> p a b", a=NX, b=S),
                ectxT[mt][:, 0:NX].unsqueeze(2).to_broadcast((128, NX, S)),
                ectxT[mt][:].unsqueeze(1).to_broadcast((128, NX, S)))
            inner = xp[:].rearrange("p (a b) -> p a b", a=NX + 4, b=PW)[
                :, 2:2 + NX, 2:2 + S]
            nc.vector.tensor_add(inner, t1[:], t2[:])
            xpad.append(xp)

        def conv(in_tiles, in_rows, wsel, oc, n_out, out_cb):
            """kt-outer 5x5 conv so weight chunks are consumed sequentially."""
            n_ic_t, n_oc_t = len(in_tiles), oc // 128
            ps_c = [T(pps, [128, n_out * S], FP32, "ps") for _ in range(n_oc_t)]
            n_acc = 25 * n_ic_t
            a = 0
            for kt in range(n_ic_t):
                for tap in range(25):
                    di, dj = divmod(tap, 5)
                    rhs = in_tiles[kt][:].rearrange(
                        "p (a b) -> p a b", a=in_rows + 4, b=PW)[
                        :, di:di + n_out, dj:dj + S]
                    w = wsel(kt, tap)
                    for mt in range(n_oc_t):
                        nc.tensor.matmul(
                            ps_c[mt][:], w[:, mt * 128:mt * 128 + 128],
                            rhs, start=(a == 0), stop=(a == n_acc - 1))
                    a += 1
            for mt in range(n_oc_t):
                out_cb(mt, ps_c[mt])

        pad1 = []
        for mt in range(2):
            t = T(pp, [128, (N1 + 4) * PW], BF16, f"pad1_{mt}")
            nc.vector.memset(t[:], 0.0)
            pad1.append(t)

        def c1_out(mt, ps):
            inner = pad1[mt][:].rearrange("p (a b) -> p a b",
                                          a=N1 + 4, b=PW)[:, 2:2 + N1, 2:2 + S]
            nc.scalar.activation(
                inner, ps[:].rearrange("p (a b) -> p a b", a=N1, b=S),
                ACT.Relu, bias=b32_t[:, B32_L["b1"] + mt:B32_L["b1"] + mt + 1])

        conv(xpad, NX, lambda kt, tap: w1_t[kt][:, tap * IC:tap * IC + IC],
             IC, N1, c1_out)

        pad2 = []
        for mt in range(2):
            t = T(pp, [128, (N2 + 4) * PW], BF16, f"pad2_{mt}")
            nc.vector.memset(t[:], 0.0)
            pad2.append(t)

        def c2_out(mt, ps):
            inner = pad2[mt][:].rearrange("p (a b) -> p a b",
                                          a=N2 + 4, b=PW)[:, 2:2 + N2, 2:2 + S]
            nc.scalar.activation(
                inner, ps[:].rearrange("p (a b) -> p a b", a=N2, b=S),
                ACT.Relu, bias=b32_t[:, B32_L["b2"] + mt:B32_L["b2"] + mt + 1])

        conv(pad1, N1, lambda kt, tap: w2_t[kt][:, tap * IC:tap * IC + IC],
             IC, N2, c2_out)

        x3 = [T(pp, [128, SP3], BF16, f"x3_{mt}") for mt in range(4)]

        def c3_out(mt, ps):
            nc.scalar.activation(
                x3[mt][:], ps[:], ACT.Relu,
                bias=b32_t[:, B32_L["b3"] + mt:B32_L["b3"] + mt + 1])

        def w3sel(kt, tap):
            half = tap // 13
            lo = (tap - 13 * half) * EMB
            return w3_t[kt * 2 + half][:, lo:lo + EMB]

        conv(pad2, N2, w3sel, EMB, N3, c3_out)

        # ---- S7: pair features + classifier ----
        x3T = []
        for i, (off, sz) in enumerate(SP3_TILES):
            t = T(pp, [sz, EMB], BF16, f"x3T{i}")
            x3T.append(t)
            for src in range(4):
                ps = T(pps, [sz, 64], FP32, "ps")
                psb = ps[:].bitcast(BF16)
                nc.tensor.transpose(psb, x3[src][:, off:off + sz],
                                    identp_t[:, :])
                nc.vector.tensor_copy(t[:, src * 128:(src + 1) * 128], psb)

        sh = c4_t[0:E, C4_L["sh"]:C4_L["sh"] + PH2]
        st = c4_t[0:E, C4_L["st"]:C4_L["st"] + PH2]
        sm = [c4_t[:, C4_L["sm"] + i * PH2:C4_L["sm"] + (i + 1) * PH2]
              for i in range(2)]

        featT = [None] * 16
        for mt in range(4):
            ps = T(pps, [128, PH2], FP32, "ps")
            nc.tensor.matmul(ps[:], ent16[:, mt * 128:(mt + 1) * 128], sh,
                             start=True, stop=True)
            t = T(pp, [128, PH2], BF16, f"featT{mt}")
            nc.vector.tensor_copy(t[:], ps[:])
            featT[mt] = t
        for mt in range(4):
            ps = T(pps, [128, PH2], FP32, "ps")
            nc.tensor.matmul(ps[:], ent16[:, mt * 128:(mt + 1) * 128], st,
                             start=True, stop=True)
            t = T(pp, [128, PH2], BF16, f"featT{4 + mt}")
            nc.vector.tensor_copy(t[:], ps[:])
            featT[4 + mt] = t
        for mt in range(4):
            ps = T(pps, [128, PH2], FP32, "ps")
            for i, (off, sz) in enumerate(SP3_TILES):
                nc.tensor.matmul(ps[:], x3T[i][:, mt * 128:(mt + 1) * 128],
                                 sm[i][0:sz, :], start=(i == 0), stop=(i == 1))
            t = T(pp, [128, PH2], BF16, f"featT{8 + mt}")
            nc.vector.tensor_copy(t[:], ps[:])
            featT[8 + mt] = t
        for mt in range(4):
            t = T(pp, [128, PH2], BF16, f"featT{12 + mt}")
            nc.vector.tensor_mul(t[:], featT[mt][:], featT[4 + mt][:])
            featT[12 + mt] = t

        ps_ht = [T(pps, [128, PH2], FP32, "ps") for _ in range(8)]
        for kt in range(16):
            w = wht_t[kt // 4]
            for mt in range(8):
                nc.tensor.matmul(
                    ps_ht[mt][:],
                    w[:, (kt % 4) * 1024 + mt * 128:(kt % 4) * 1024 + mt * 128 + 128],
                    featT[kt][:], start=(kt == 0), stop=(kt == 15))
        htT = []
        for mt in range(8):
            t = T(pp, [128, PH2], BF16, f"htT{mt}")
            nc.scalar.activation(
                t[:], ps_ht[mt][:], ACT.Tanh,
                bias=b32_t[:, B32_L["bht"] + mt:B32_L["bht"] + mt + 1])
            htT.append(t)

        ps_out = T(pps, [97, PH2], FP32, "ps")
        for kt in range(8):
            nc.tensor.matmul(ps_out[:], wbil_t[:, kt * 97:(kt + 1) * 97],
                             htT[kt][:], start=(kt == 0), stop=(kt == 7))
        out_sb = T(pp, [97, PH2], FP32, "out")
        nc.vector.tensor_scalar_add(
            out_sb[:], ps_out[:],
            b32_t[0:97, B32_L["bbil"]:B32_L["bbil"] + 1])
        dma(outt, out_sb[:])

    nc.compile()
    return nc


_PROG = None


def _get_prog():
    global _PROG
    if _PROG is None:
        _PROG = build_program()
    return _PROG


def _padtiles(arr, tile_rows=128):
    """[R, W] -> [128, ceil(R/128)*W] column-blocked, rows zero-padded."""
    r, w = arr.shape
    nt = (r + tile_rows - 1) // tile_rows
    out = np.zeros((tile_rows, nt * w), arr.dtype)
    for i in range(nt):
        blk = arr[i * tile_rows:(i + 1) * tile_rows]
        out[:blk.shape[0], i * w:i * w + w] = blk
    return out


def _shared_inputs(inputs):
    f32 = np.float32
    bf = ml_dtypes.bfloat16
    sh = {}
    sh["identp"] = np.eye(128, dtype=bf)
    # blob32
    b32 = np.zeros((128, B32_W), f32)
    b32[:, B32_L["ident"]:B32_L["ident"] + 128] = np.eye(128, dtype=f32)
    b32[:, B32_L["ones"]] = 1.0
    b32[0, B32_L["onesrow"]:B32_L["onesrow"] + 128] = 1.0
    b32[0, B32_L["btrans"]:B32_L["btrans"] + 512] = np.asarray(
        inputs["b_trans"], f32)
    b32[0, B32_L["brgcn"]:B32_L["brgcn"] + 512] = np.asarray(
        inputs["b_rgcn"], f32)
    b32[:, B32_L["b1"]:B32_L["b1"] + 2] = np.asarray(
        inputs["conv1_b"], f32).reshape(2, 128).T
    b32[:, B32_L["b2"]:B32_L["b2"] + 2] = np.asarray(
        inputs["conv2_b"], f32).reshape(2, 128).T
    b32[:, B32_L["b3"]:B32_L["b3"] + 4] = np.asarray(
        inputs["conv3_b"], f32).reshape(4, 128).T
    b32[:, B32_L["bht"]:B32_L["bht"] + 8] = np.asarray(
        inputs["ht_b"], f32).reshape(8, 128).T
    b32[0:97, B32_L["bbil"]] = np.asarray(inputs["bil_b"], f32)
    sh["b32"] = b32
    # wrel: 25 blocks (r*5+i) of [<=128, 512], 13 blocks per WCH chunk
    wcat = np.concatenate(
        [np.asarray(inputs["W_rel"], f32).reshape(4 * NF, EMB),
         np.asarray(inputs["W_self"], f32)], axis=0)
    wr = np.zeros((128, 2 * WCH), f32)
    for r in range(5):
        for i in range(5):
            blk = wcat[r * NF + i * 128:r * NF + min((i + 1) * 128, NF)]
            bidx = r * 5 + i
            col = (bidx // 13) * WCH + (bidx % 13) * 512
            wr[:blk.shape[0], col:col + 512] = blk
    sh["wrel"] = wr.astype(bf)
    # conv weights, two spatial variants
    def convpack(w, nkt, oc, nchunk):
        # w [oc, ic, 5, 5] -> [128, nchunk*WCH]; within a chunk, col tap*oc + o
        t = np.ascontiguousarray(np.asarray(w, f32).transpose(1, 2, 3, 0))
        t = t.reshape(nkt, 128, 25, oc).transpose(1, 0, 2, 3)  # [128,kt,tap,oc]
        out = np.zeros((128, nchunk * WCH), f32)
        taps_per = 25 // (nchunk // nkt) if nchunk != nkt else 25
        if nchunk == nkt:           # whole kt fits one chunk (25*oc <= WCH)
            for kt in range(nkt):
                out[:, kt * WCH:kt * WCH + 25 * oc] = t[:, kt].reshape(128, -1)
        else:                       # kt split into 2 chunks: taps 0-12 / 13-24
            for kt in range(nkt):
                for half, (t0, t1) in enumerate([(0, 13), (13, 25)]):
                    ch = kt * 2 + half
                    blk = t[:, kt, t0:t1].reshape(128, -1)
                    out[:, ch * WCH:ch * WCH + blk.shape[1]] = blk
        return np.ascontiguousarray(out).astype(bf)
    for hh in range(2):
        w1 = np.asarray(inputs["conv1_w"], f32)
        w2 = np.asarray(inputs["conv2_w"], f32)
        w3 = np.asarray(inputs["conv3_w"], f32)
        if hh == 1:
            w1 = w1[:, :, ::-1, ::-1]
            w2 = w2[:, :, ::-1, ::-1]
            w3 = w3[:, :, ::-1, ::-1]
        sh[f"w1_{hh}"] = convpack(w1, 4, IC, 4)
        sh[f"w2_{hh}"] = convpack(w2, 2, IC, 2)
        sh[f"w3_{hh}"] = convpack(w3, 2, EMB, 4)
    sh["wht"] = np.ascontiguousarray(
        np.asarray(inputs["ht_W"], f32).reshape(16, 128, 1024)
        .transpose(1, 0, 2).reshape(128, 16 * 1024)).astype(bf)
    sh["wbil"] = np.ascontiguousarray(
        np.asarray(inputs["bil_W"], f32).reshape(8, 128, 97)
        .transpose(1, 0, 2).reshape(128, 8 * 97)).astype(bf)
    # structural constants
    sh["_g3"] = np.kron(np.eye(E, dtype=f32), np.ones((M, 1), f32))
    sh["_gmat"] = np.kron(np.eye(E, dtype=f32),
                          np.ones((M * NH, 1), f32) / (M * NH))
    sh["_gspan"] = np.kron(np.eye(L, dtype=f32), np.ones((LS, 1), f32))
    sh["_wtrans"] = np.asarray(inputs["W_trans"], f32)
    sh["_type_embed"] = np.asarray(inputs["type_embed"], f32)
    return sh


def _core_inputs(inputs, shared, b, hh):
    f32 = np.float32
    bf = ml_dtypes.bfloat16
    if hh == 0:
        perm_e = np.arange(E)
    else:
        perm_e = np.arange(E)[::-1]
    perm_m = (perm_e[:, None] * M + np.arange(M)).reshape(-1)
    p_nodes = np.concatenate([perm_e, E + perm_m, np.arange(E + E * M, NN)])
    X = np.asarray(inputs["sequence_output"][b], f32)
    att = np.asarray(inputs["attention"][b], f32)
    adj = np.asarray(inputs["adjacency"][b], f32)[:, p_nodes][:, :, p_nodes]
    mf = np.asarray(inputs["mention_idx"][b]).astype(np.int64)[perm_e].reshape(-1)
    ls = np.asarray(inputs["link_start"][b]).astype(np.int64)
    ntypes = np.asarray(inputs["node_types"][b]).astype(np.int64)[p_nodes]
    hts = np.asarray(inputs["hts"][b]).astype(np.int64)

    # C0
    c0 = np.zeros((128, C0_W), bf)
    adjrel = np.zeros((NN, 110), f32)
    for r in range(4):
        adjrel[:, r * 22:(r + 1) * 22] = adj[r].T[:, :E]
    adjrel[:, 88:110] = np.eye(NN, dtype=f32)[:, :E]
    typ_all = shared["_type_embed"][ntypes]
    goff = 0
    for gi, g in enumerate([22, 66, 30]):
        c0[0:g, C0_L[f"adjrel{gi}"]:C0_L[f"adjrel{gi}"] + 110] = \
            adjrel[goff:goff + g].astype(bf)
        c0[0:g, C0_L[f"typ{gi}"]:C0_L[f"typ{gi}"] + 20] = \
            typ_all[goff:goff + g].astype(bf)
        goff += g
    c0[0:E * M, C0_L["g3"]:C0_L["g3"] + 22] = shared["_g3"].astype(bf)
    c0[:, C0_L["ones16"]] = 1.0
    c0[:, C0_L["gmat"]:C0_L["gmat"] + GM_TILES * 22] = _padtiles(
        shared["_gmat"].astype(bf))
    c0[:, C0_L["gspan"]:C0_L["gspan"] + SPAN_TILES * 30] = _padtiles(
        shared["_gspan"].astype(bf))
    pos = ls[:, None] + np.arange(LS)
    attl = np.empty((L * LS, NH * LS), f32)
    for l in range(L):
        blk = att[:, pos[l]][:, :, pos[l]]
        attl[l * LS:(l + 1) * LS] = blk.transpose(2, 0, 1).reshape(LS, NH * LS)
    c0[:, C0_L["attl"]:C0_L["attl"] + SPAN_TILES * 192] = _padtiles(
        attl.astype(bf))
    c0[:, C0_L["xg"]:C0_L["xg"] + 6 * 66] = _padtiles(
        np.ascontiguousarray(X[mf].T).astype(bf))
    c0[:, C0_L["wtrans"]:C0_L["wtrans"] + 6 * 512] = _padtiles(
        shared["_wtrans"].astype(bf))
    # C1 xspan, C2 attm, C3 xn
    c1 = _padtiles(X[pos.reshape(-1)].astype(bf))
    attm = np.ascontiguousarray(
        att[:, mf].transpose(1, 0, 2).reshape(E * M * NH, C))
    c2 = np.zeros((128, 8192), bf)
    c2[:, :C2_W] = _padtiles(attm.astype(bf))
    c3 = _padtiles(X.astype(bf))
    # C4 selectors (local band: h' in 0..10)
    inv = np.empty(E, np.int64)
    inv[perm_e] = np.arange(E)
    hl, tl = inv[hts[:, 0]], inv[hts[:, 1]]
    sel = np.where(hl <= 10)[0]
    assert len(sel) <= PH2, f"band overflow: {len(sel)} > {PH2}"
    c4 = np.zeros((128, C4_W), bf)
    shm = np.zeros((E, PH2), f32)
    shm[hl[sel], np.arange(len(sel))] = 1.0
    stm = np.zeros((E, PH2), f32)
    stm[tl[sel], np.arange(len(sel))] = 1.0
    smm = np.zeros((SP3, PH2), f32)
    smm[hl[sel] * S + tl[sel], np.arange(len(sel))] = 1.0
    c4[0:E, C4_L["sh"]:C4_L["sh"] + PH2] = shm.astype(bf)
    c4[0:E, C4_L["st"]:C4_L["st"] + PH2] = stm.astype(bf)
    c4[:, C4_L["sm"]:C4_L["sm"] + 2 * PH2] = _padtiles(smm.astype(bf))

    m = {
        "identp": shared["identp"], "b32": shared["b32"],
        "wrel": shared["wrel"], "wht": shared["wht"], "wbil": shared["wbil"],
        "w1": shared[f"w1_{hh}"], "w2": shared[f"w2_{hh}"],
        "w3": shared[f"w3_{hh}"],
        "c0": c0, "c1": c1, "c2": c2, "c3": c3, "c4": c4,
    }
    return m, sel


def kernel(**inputs):
    nc = _get_prog()
    shared = _shared_inputs(inputs)
    in_maps, sels = [], []
    for b in range(B):
        for hh in range(2):
            m, sel = _core_inputs(inputs, shared, b, hh)
            in_maps.append(m)
            sels.append(sel)
    res = run_bass_kernel_spmd(nc, in_maps, core_ids=list(range(8)))
    out = np.empty((B, P, 97), np.float32)
    for b in range(B):
        for hh in range(2):
            ci = 2 * b + hh
            sel = sels[ci]
            r = np.asarray(res.results[ci]["outt"], np.float32)
            out[b, sel, :] = r[:, :len(sel)].T
    return out
